# revision 1
# baseline (speedup 1.0000x reference)
# Trainium2 Bass kernel for nn_DecoderBlock (B=4, T=2048, E=1024, H=16, D=64, FF=4096).
#
# Sharding: 8-way data parallel, zero collectives. Core c = 2*b + h handles batch b
# and the interleaved half of the sequence: 128-row q-blocks {2s+h : s=0..7}
# (1024 q rows per core). K/V are computed per-core for the full T=2048 rows of its
# batch (duplicated across the two cores of a batch pair) so attention needs no
# cross-core communication. The interleaved block assignment makes the causal
# work pattern identical on every core (uniform SPMD program): q-slot s statically
# attends keys [0, 256*(s+1)), with a per-core {0,1} multiplicative mask (input
# data) handling the parity-dependent diagonal.
#
# On-chip layout: activations flow feature-major (S^T = [k, q]) through attention so
# softmax needs no transposes of the probability matrix. Softmax uses no max
# subtraction (scores are ~N(0, 0.25^2) by construction); 1/denominator is
# exp(-ln(d)) so the whole kernel uses one ACT table set (exp/ln/relu).
# Matmuls run in bf16 with fp32 PSUM accumulation; LN statistics, residuals and
# the final output stay fp32. LN gains (g1,g2) and the attention 1/sqrt(E) scale
# are folded into the weights on the host; beta terms become per-feature biases.

import numpy as np
import ml_dtypes
from contextlib import ExitStack

BF16 = ml_dtypes.bfloat16

B, T, E, H, D, FF = 4, 2048, 1024, 16, 64, 4096
M = 1024          # q rows per core
NCORES = 8
NS = 8            # q slots (128 rows) per core
ET = E // 128     # 8 e-tiles
TK = T // 128     # 16 k-tiles
FT = FF // 128    # 32 ff-tiles
NP = H // 2       # 8 head pairs
EPS = 1e-5

_CACHE = {}


def _build(repeat=1):
    """Build (and cache) the Bass module for one core's uniform program.

    repeat>1 emits the whole body N times (identical I/O) — used only for
    slope-based wall-clock timing of one body on hardware.
    """
    key = ("nc", repeat)
    if key in _CACHE:
        return _CACHE[key]

    import concourse.bacc as bacc
    import concourse.tile as tile
    import concourse.mybir as mybir
    from concourse import masks as cmasks

    dt = mybir.dt
    f32, bf16 = dt.float32, dt.bfloat16
    AF = mybir.ActivationFunctionType
    OP = mybir.AluOpType

    nc = bacc.Bacc("TRN2", target_bir_lowering=False, debug=False,
                   num_devices=NCORES)

    # Every activation we use (Exp, Ln, Relu, Copy, Identity) lives in the
    # 'natural_log_exp_and_others' table set. The default per-function set
    # choice alternates home sets (exp_and_others vs natural_log), inserting
    # ~80 ACT table loads (~100us). Restrict the chooser to the one set that
    # covers everything -> a single load.
    import types
    import bass_rust as _br

    def _insert_act_loads_one_set(self):
        has_activation = any(
            isinstance(i, mybir.InstActivation)
            for b in self.main_func.blocks for i in b.instructions)
        if not has_activation:
            return
        tabs = bacc.get_activation_tables(self.m.arch)
        ours = {mybir.ActivationFunctionType.Exp, mybir.ActivationFunctionType.Ln,
                mybir.ActivationFunctionType.Relu, mybir.ActivationFunctionType.Copy,
                mybir.ActivationFunctionType.Identity}
        filt = []
        for name, fns in tabs.items():
            if name == "natural_log_exp_and_others":
                assert ours <= fns
                filt.append((name, fns))
            else:
                filt.append((name, fns - ours))
        _br.insert_act_table_loads(self, filt)

    nc.insert_act_table_loads = types.MethodType(_insert_act_loads_one_set, nc)

    # ----- DRAM I/O -----
    x_full = nc.dram_tensor("x_full", [T, E], f32, kind="ExternalInput").ap()
    x_q = nc.dram_tensor("x_q", [M, E], f32, kind="ExternalInput").ap()
    xqp = nc.dram_tensor("xqp", [M, E], f32, kind="ExternalInput").ap()
    # weights arrive pre-arranged on the host into SBUF layout
    # [128 partitions, <tile dims>] so each DMA is one long contiguous run
    # per partition (minimal descriptor count).
    wq = nc.dram_tensor("wq", [128, ET * H * D], bf16, kind="ExternalInput").ap()
    wk = nc.dram_tensor("wk", [128, ET * H * D], bf16, kind="ExternalInput").ap()
    wv = nc.dram_tensor("wv", [128, ET * H * D], bf16, kind="ExternalInput").ap()
    projw = nc.dram_tensor("projw", [128, NP * E], bf16, kind="ExternalInput").ap()
    w1 = nc.dram_tensor("w1", [128, FT * ET * 128], bf16,
                        kind="ExternalInput").ap()
    w2 = nc.dram_tensor("w2", [128, FT * E], bf16, kind="ExternalInput").ap()
    qb_d = nc.dram_tensor("qb", [128, ET], f32, kind="ExternalInput").ap()
    kb_d = nc.dram_tensor("kb", [128, ET], f32, kind="ExternalInput").ap()
    vb_d = nc.dram_tensor("vb", [128, H * D], f32, kind="ExternalInput").ap()
    b1_d = nc.dram_tensor("b1", [128, FT], f32, kind="ExternalInput").ap()
    bf2_d = nc.dram_tensor("bf2b", [128, E], f32, kind="ExternalInput").ap()
    maskE_d = nc.dram_tensor("maskE", [128, 256], bf16, kind="ExternalInput").ap()
    maskO_d = nc.dram_tensor("maskO", [128, 256], bf16, kind="ExternalInput").ap()
    out = nc.dram_tensor("out", [M, E], f32, kind="ExternalOutput").ap()

    with tile.TileContext(nc) as tc:
      for _rep in range(repeat):
        es = ExitStack()
        with es:
            # ---------- constants (whole kernel) ----------
            constp = es.enter_context(tc.tile_pool(name="const", bufs=1))
            ident = constp.tile([128, 128], bf16)
            cmasks.make_identity(nc, ident[:])
            ones64 = constp.tile([128, 64], bf16)
            nc.gpsimd.memset(ones64[:], 1.0)
            maskE = constp.tile([128, 256], bf16)
            nc.sync.dma_start(maskE[:], maskE_d)
            maskO = constp.tile([128, 256], bf16)
            nc.sync.dma_start(maskO[:], maskO_d)
            qb = constp.tile([128, ET], f32)
            nc.sync.dma_start(qb[:], qb_d)
            kb = constp.tile([128, ET], f32)
            nc.sync.dma_start(kb[:], kb_d)
            vb = constp.tile([128, H * D], f32)
            nc.sync.dma_start(vb[:], vb_d)
            b1 = constp.tile([128, FT], f32)
            nc.sync.dma_start(b1[:], b1_d)
            bf2 = constp.tile([128, E], f32)
            nc.sync.dma_start(bf2[:], bf2_d)
            eps_t = constp.tile([128, 1], f32)
            nc.gpsimd.memset(eps_t[:], EPS)

            # helper: layernorm one 128-row chunk (fp32 src slice in SBUF) and
            # write the transposed bf16 result into dst_T[:, et, col:col+128].
            def ln_stats(src, statp):
                st = statp.tile([128, 2, 6], f32, tag="st")
                for g in range(2):
                    nc.vector.bn_stats(st[:, g, :], src[:, g * 512:(g + 1) * 512])
                ag = statp.tile([128, 2], f32, tag="ag")
                nc.vector.bn_aggr(ag[:], st[:])
                lv = statp.tile([128, 1], f32, tag="lv")
                nc.scalar.activation(lv[:], ag[:, 1:2], AF.Ln, bias=eps_t[:])
                rstd = statp.tile([128, 1], f32, tag="rstd")
                nc.scalar.activation(rstd[:], lv[:], AF.Exp, scale=-0.5)
                return ag, rstd

            def ln_chunk(src, dst_T, col, statp, lnstage, tpsum, ci,
                         stats=None):
                ag, rstd = stats if stats is not None else ln_stats(src, statp)
                lc = lnstage.tile([128, E], bf16)
                # split the normalize-apply across DVE and GPSIMD so the
                # per-chunk chain latency halves and both engines share work
                nc.vector.tensor_scalar(lc[:, 0:512], src[:, 0:512],
                                        ag[:, 0:1], rstd[:],
                                        OP.subtract, OP.mult)
                nc.gpsimd.tensor_scalar(lc[:, 512:1024], src[:, 512:1024],
                                        ag[:, 0:1], rstd[:],
                                        OP.subtract, OP.mult)
                import os as _os
                if _os.environ.get("KT_DMA_TRANSPOSE", "0") == "1":
                    for et in range(ET):
                        nc.sync.dma_start(dst_T[:, et, col:col + 128],
                                            lc[:, et * 128:(et + 1) * 128],
                                            transpose=True)
                else:
                    for et in range(ET):
                        tp = tpsum.tile([128, 128], bf16)
                        nc.tensor.transpose(tp[:],
                                            lc[:, et * 128:(et + 1) * 128],
                                            ident[:])
                        dst = dst_T[:, et, col:col + 128]
                        if (et + ci) % 2 == 0:
                            nc.vector.tensor_copy(dst, tp[:])
                        else:
                            nc.scalar.copy(dst, tp[:])
                        # (copies stay off GPSIMD: it cannot read PSUM)

            # ---------- scope B: qT/kT/v (strict stack nesting) ----------
            xmid = es.enter_context(tc.tile_pool(name="xmidp", bufs=1)).tile(
                [128, NS, E], f32)
            with ExitStack() as sB:
                qT = sB.enter_context(tc.tile_pool(name="qTp", bufs=1)).tile(
                    [128, NP, M], bf16)
                kT = sB.enter_context(tc.tile_pool(name="kTp", bufs=1)).tile(
                    [128, NP, T], bf16)
                vS = sB.enter_context(tc.tile_pool(name="vp", bufs=1)).tile(
                    [128, TK, H * D], bf16)

                # ---------- scope A: LN1 + QKV projections ----------
                with ExitStack() as sA:
                    wpool = sA.enter_context(tc.tile_pool(name="wpool", bufs=1))
                    stage = sA.enter_context(tc.tile_pool(name="xstage", bufs=3))
                    lnstage = sA.enter_context(tc.tile_pool(name="lnstage", bufs=3))
                    statp = sA.enter_context(tc.tile_pool(name="statp", bufs=6))
                    tpsum = sA.enter_context(
                        tc.tile_pool(name="tpsum", bufs=4, space="PSUM"))
                    qps = sA.enter_context(
                        tc.tile_pool(name="qps", bufs=2, space="PSUM"))

                    with tc.tile_pool(name="lnqp", bufs=1) as lnqp:
                        lnq = lnqp.tile([128, ET, M], bf16)
                        # LN1 on the gathered q rows -> lnq (e-major)
                        for s in range(NS):
                            xc = stage.tile([128, E], f32)
                            nc.scalar.dma_start(xc[:], x_q[s * 128:(s + 1) * 128, :])
                            ln_chunk(xc[:], lnq, s * 128, statp, lnstage, tpsum, s)

                        # pre-issue DMA+stats of the first x_full chunks so
                        # their normalize-applies are ready right after Q^T
                        pre = []
                        for c in range(2):
                            xc = stage.tile([128, E], f32)
                            nc.scalar.dma_start(
                                xc[:], x_full[c * 128:(c + 1) * 128, :])
                            pre.append((xc, ln_stats(xc[:], statp)))

                        # Q^T = (wq)^T @ lnq^T   [hd, q]
                        wq_sb = wpool.tile([128, ET, H * D], bf16, tag="w")
                        nc.sync.dma_start(wq_sb[:], wq.rearrange(
                            "p (et n) -> p et n", et=ET))
                        for m in range(ET):
                            ps = qps.tile([128, 1024], f32)
                            for qc in range(2):
                                for et in range(ET):
                                    nc.tensor.matmul(
                                        ps[:, qc * 512:(qc + 1) * 512],
                                        lhsT=wq_sb[:, et, m * 128:(m + 1) * 128],
                                        rhs=lnq[:, et, qc * 512:(qc + 1) * 512],
                                        start=(et == 0), stop=(et == ET - 1))
                            nc.vector.tensor_scalar_add(
                                qT[:, m, :], ps[:], qb[:, m:m + 1])

                    lnf = sA.enter_context(tc.tile_pool(name="lnfp", bufs=1)).tile(
                        [128, ET, T], bf16)
                    # LN1 on the full batch rows -> lnf
                    for c in range(TK):
                        if c < len(pre):
                            xc, stats = pre[c]
                            ln_chunk(xc[:], lnf, c * 128, statp, lnstage,
                                     tpsum, c, stats=stats)
                        else:
                            xc = stage.tile([128, E], f32)
                            nc.scalar.dma_start(
                                xc[:], x_full[c * 128:(c + 1) * 128, :])
                            ln_chunk(xc[:], lnf, c * 128, statp, lnstage,
                                     tpsum, c)

                    # K^T [hd, t]
                    wk_sb = wpool.tile([128, ET, H * D], bf16, tag="w")
                    nc.sync.dma_start(wk_sb[:], wk.rearrange(
                        "p (et n) -> p et n", et=ET))
                    for m in range(ET):
                        for kh in range(2):
                            ps = qps.tile([128, 1024], f32)
                            for kc in range(2 * kh, 2 * kh + 2):
                                for et in range(ET):
                                    nc.tensor.matmul(
                                        ps[:, (kc % 2) * 512:(kc % 2 + 1) * 512],
                                        lhsT=wk_sb[:, et, m * 128:(m + 1) * 128],
                                        rhs=lnf[:, et, kc * 512:(kc + 1) * 512],
                                        start=(et == 0), stop=(et == ET - 1))
                            nc.vector.tensor_scalar_add(
                                kT[:, m, kh * 1024:(kh + 1) * 1024], ps[:],
                                kb[:, m:m + 1])

                    # V [t, hd] (token-major)
                    wv_sb = wpool.tile([128, ET, H * D], bf16, tag="w")
                    nc.sync.dma_start(wv_sb[:], wv.rearrange(
                        "p (et n) -> p et n", et=ET))
                    for t in range(TK):
                        ps = qps.tile([128, 1024], f32)
                        for hc in range(2):
                            for et in range(ET):
                                nc.tensor.matmul(
                                    ps[:, hc * 512:(hc + 1) * 512],
                                    lhsT=lnf[:, et, t * 128:(t + 1) * 128],
                                    rhs=wv_sb[:, et, hc * 512:(hc + 1) * 512],
                                    start=(et == 0), stop=(et == ET - 1))
                        nc.vector.tensor_add(vS[:, t, :], ps[:], vb[:])

                # ---------- attention + output projection ----------
                with ExitStack() as sC:
                    oT = sC.enter_context(tc.tile_pool(name="oTp", bufs=1)).tile(
                        [128, NP, M], bf16)

                    with ExitStack() as sAtt:
                        ptp = sAtt.enter_context(tc.tile_pool(name="ptp", bufs=6))
                        normp = sAtt.enter_context(
                            tc.tile_pool(name="normp", bufs=3))
                        apsum = sAtt.enter_context(
                            tc.tile_pool(name="apsum", bufs=1, space="PSUM"))
                        spsum = sAtt.enter_context(
                            tc.tile_pool(name="spsum", bufs=2, space="PSUM"))

                        maskEv = maskE[:].rearrange("p (h q) -> p h q", h=2)
                        maskOv = maskO[:].rearrange("p (h q) -> p h q", h=2)

                        for p in range(NP):
                            av = apsum.tile([128, M], f32, tag="av")
                            den = apsum.tile([128, M], f32, tag="den")
                            pend = []  # software-pipelined AV work

                            def do_av(item, av=av, den=den, p=p):
                                half, kt, qlo, pt = item
                                colr = slice(512 * half + qlo, 512 * (half + 1))
                                st = (kt == 0)
                                sp = (kt == (7 if half == 0 else 15))
                                for h in range(2):
                                    hd = (2 * p + h) * 64
                                    nc.tensor.matmul(
                                        av[64 * h:64 * h + 64, colr],
                                        lhsT=vS[:, kt, hd:hd + 64],
                                        rhs=pt[:, h, qlo:512],
                                        start=st, stop=sp, skip_group_check=True)
                                    nc.tensor.matmul(
                                        den[64 * h:64 * h + 64, colr],
                                        lhsT=ones64[:],
                                        rhs=pt[:, h, qlo:512],
                                        start=st, stop=sp, skip_group_check=True)

                            for half in range(2):
                                for kt in range(8 if half == 0 else 16):
                                    qlo = max(0, 128 * (kt // 2) - 512 * half)
                                    ps = spsum.tile([128, 2, 512], f32)
                                    for h in range(2):
                                        nc.tensor.matmul(
                                            ps[:, h, qlo:512],
                                            lhsT=kT[64 * h:64 * h + 64, p,
                                                    kt * 128:(kt + 1) * 128],
                                            rhs=qT[64 * h:64 * h + 64, p,
                                                   512 * half + qlo:
                                                   512 * (half + 1)],
                                            start=True, stop=True)
                                    pt = ptp.tile([128, 2, 512], bf16)
                                    nc.scalar.activation(
                                        pt[:, :, qlo:512], ps[:, :, qlo:512],
                                        AF.Exp)
                                    if (kt // 2) >= 4 * half:
                                        mk = maskEv if kt % 2 == 0 else maskOv
                                        nc.vector.tensor_mul(
                                            pt[:, :, qlo:qlo + 128],
                                            pt[:, :, qlo:qlo + 128], mk)
                                    pend.append((half, kt, qlo, pt))
                                    if len(pend) > 4:
                                        do_av(pend.pop(0))
                            for item in pend:
                                do_av(item)
                            tln = normp.tile([128, M], f32, tag="tln")
                            nc.scalar.activation(tln[:], den[:], AF.Ln)
                            rcp = normp.tile([128, M], bf16, tag="rcp")
                            nc.scalar.activation(rcp[:], tln[:], AF.Exp,
                                                 scale=-1.0)
                            nc.vector.tensor_mul(oT[:, p, :], av[:], rcp[:])

                    # output projection + residual -> xmid (fp32)
                    with ExitStack() as sProj:
                        xqps = sProj.enter_context(
                            tc.tile_pool(name="xqpp", bufs=1)).tile(
                            [128, NS, E], f32)
                        for s in range(NS):
                            nc.sync.dma_start(
                                xqps[:, s, :], xqp[s * 128:(s + 1) * 128, :])
                        pw_sb = sProj.enter_context(
                            tc.tile_pool(name="pwp", bufs=1)).tile(
                            [128, NP, E], bf16)
                        nc.sync.dma_start(pw_sb[:], projw.rearrange(
                            "p (m e) -> p m e", m=NP))
                        pps = sProj.enter_context(
                            tc.tile_pool(name="pps", bufs=3, space="PSUM"))
                        for qm in range(NS):
                            ps = pps.tile([128, 1024], f32)
                            for ec in range(2):
                                for pk in range(NP):
                                    nc.tensor.matmul(
                                        ps[:, ec * 512:(ec + 1) * 512],
                                        lhsT=oT[:, pk, qm * 128:(qm + 1) * 128],
                                        rhs=pw_sb[:, pk, ec * 512:(ec + 1) * 512],
                                        start=(pk == 0), stop=(pk == NP - 1))
                            nc.vector.tensor_add(
                                xmid[:, qm, :], ps[:], xqps[:, qm, :])

            # ---------- scope D: LN2 + FFN ----------
            with ExitStack() as sD:
                ln2T = sD.enter_context(tc.tile_pool(name="ln2p", bufs=1)).tile(
                    [128, ET, M], bf16)
                w2_sb = sD.enter_context(tc.tile_pool(name="w2p", bufs=1)).tile(
                    [128, FT, E], bf16)
                nc.sync.dma_start(w2_sb[:], w2.rearrange(
                    "p (ft e) -> p ft e", ft=FT))

                with ExitStack() as sLN2:
                    statp2 = sLN2.enter_context(tc.tile_pool(name="statp2", bufs=6))
                    lnstage2 = sLN2.enter_context(
                        tc.tile_pool(name="lnstage2", bufs=3))
                    tpsum2 = sLN2.enter_context(
                        tc.tile_pool(name="tpsum2", bufs=3, space="PSUM"))
                    for qm in range(NS):
                        ln_chunk(xmid[:, qm, :], ln2T, qm * 128, statp2,
                                 lnstage2, tpsum2, qm)
                        # after LN2 consumed xmid, fold the final bf2 bias in
                        nc.vector.tensor_add(xmid[:, qm, :], xmid[:, qm, :],
                                             bf2[:])

                rtp = sD.enter_context(tc.tile_pool(name="rtp", bufs=1))
                w1p = sD.enter_context(tc.tile_pool(name="w1p", bufs=4))
                zps = sD.enter_context(
                    tc.tile_pool(name="zps", bufs=2, space="PSUM"))
                ops = sD.enter_context(
                    tc.tile_pool(name="ops", bufs=2, space="PSUM"))
                outp = sD.enter_context(tc.tile_pool(name="outp", bufs=3))

                for half in range(2):
                    rT = rtp.tile([128, FT, 512], bf16, tag="rT")
                    for fm in range(FT):
                        w1f = w1p.tile([128, ET, 128], bf16)
                        nc.sync.dma_start(
                            w1f[:], w1.rearrange("p (fm et f) -> p fm et f",
                                                 fm=FT, et=ET)[:, fm])
                        zp = zps.tile([128, 512], f32)
                        for et in range(ET):
                            nc.tensor.matmul(
                                zp[:],
                                lhsT=w1f[:, et, :],
                                rhs=ln2T[:, et, half * 512:(half + 1) * 512],
                                start=(et == 0), stop=(et == ET - 1))
                        nc.scalar.activation(rT[:, fm, :], zp[:], AF.Relu,
                                             bias=b1[:, fm:fm + 1])
                    for qq in range(4):
                        qm = half * 4 + qq
                        ot = outp.tile([128, E], f32)
                        op = ops.tile([128, 1024], f32)
                        for ec in range(2):
                            for fk in range(FT):
                                nc.tensor.matmul(
                                    op[:, ec * 512:(ec + 1) * 512],
                                    lhsT=rT[:, fk, qq * 128:(qq + 1) * 128],
                                    rhs=w2_sb[:, fk, ec * 512:(ec + 1) * 512],
                                    start=(fk == 0), stop=(fk == FT - 1))
                        nc.vector.tensor_add(ot[:], op[:], xmid[:, qm, :])
                        nc.scalar.dma_start(out[qm * 128:(qm + 1) * 128, :], ot[:])

    nc.compile()
    _CACHE[key] = nc
    return nc


def _prep_inputs(x, wq, wk, wv, proj_w, proj_b, g1, beta1, g2, beta2, w1, bf1,
                 w2, bf2):
    """Host-side sharding + weight folding. Returns list of 8 in_maps."""
    f32 = np.float32
    x = np.asarray(x, f32)
    scale = float(E) ** -0.5

    Wq = np.asarray(wq, f32).transpose(1, 0, 2).reshape(E, H * D) * scale
    Wk = np.asarray(wk, f32).transpose(1, 0, 2).reshape(E, H * D)
    Wv = np.asarray(wv, f32).transpose(1, 0, 2).reshape(E, H * D)
    g1 = np.asarray(g1, f32)
    beta1 = np.asarray(beta1, f32)
    g2 = np.asarray(g2, f32)
    beta2 = np.asarray(beta2, f32)
    w1 = np.asarray(w1, f32)
    w2 = np.asarray(w2, f32)
    bf1 = np.asarray(bf1, f32)
    bf2 = np.asarray(bf2, f32)
    proj_w = np.asarray(proj_w, f32)
    proj_b = np.asarray(proj_b, f32)

    def sb_layout(w, ntile):
        # [ntile*128, N] -> [128, ntile*N] with per-partition contiguous tiles
        n = w.shape[1]
        return np.ascontiguousarray(
            w.reshape(ntile, 128, n).transpose(1, 0, 2).reshape(128, ntile * n))

    wq_b = sb_layout((Wq * g1[:, None]).astype(BF16), ET)
    wk_b = sb_layout((Wk * g1[:, None]).astype(BF16), ET)
    wv_b = sb_layout((Wv * g1[:, None]).astype(BF16), ET)
    qbias = beta1 @ Wq
    kbias = beta1 @ Wk
    vbias = beta1 @ Wv
    w1_b = np.ascontiguousarray(
        (w1 * g2[:, None]).astype(BF16)
        .reshape(ET, 128, FT, 128).transpose(1, 2, 0, 3)
        .reshape(128, FT * ET * 128))
    b1v = bf1 + beta2 @ w1
    w2_b = sb_layout(w2.astype(BF16), FT)
    projw_b = sb_layout(proj_w.astype(BF16), NP)

    qb = np.ascontiguousarray(qbias.reshape(ET, 128).T, f32)
    kb = np.ascontiguousarray(kbias.reshape(ET, 128).T, f32)
    vb = np.ascontiguousarray(np.broadcast_to(vbias, (128, H * D)), f32)
    b1m = np.ascontiguousarray(b1v.reshape(FT, 128).T, f32)
    bf2m = np.ascontiguousarray(np.broadcast_to(bf2, (128, E)), f32)

    tri = np.triu(np.ones((128, 128), f32))  # [k_row, q_col]: 1 iff k <= q
    onesm = np.ones((128, 128), f32)
    zerosm = np.zeros((128, 128), f32)
    mE = {0: tri, 1: onesm}
    mO = {0: zerosm, 1: tri}

    in_maps = []
    for c in range(NCORES):
        b, hpar = c // 2, c % 2
        xb = x[b]
        xq = np.ascontiguousarray(
            xb.reshape(TK, 128, E)[hpar::2].reshape(M, E), f32)
        in_maps.append({
            "x_full": np.ascontiguousarray(xb, f32),
            "x_q": xq,
            "xqp": xq + proj_b[None, :].astype(f32),
            "wq": wq_b, "wk": wk_b, "wv": wv_b,
            "projw": projw_b, "w1": w1_b, "w2": w2_b,
            "qb": qb, "kb": kb, "vb": vb, "b1": b1m, "bf2b": bf2m,
            "maskE": np.ascontiguousarray(
                np.tile(mE[hpar], (1, 2))).astype(BF16),
            "maskO": np.ascontiguousarray(
                np.tile(mO[hpar], (1, 2))).astype(BF16),
        })
    return in_maps


def _run(inputs, trace=False):
    from concourse.bass_utils import run_bass_kernel_spmd
    nc = _build()
    in_maps = _prep_inputs(**inputs)
    res = run_bass_kernel_spmd(nc, in_maps, core_ids=list(range(NCORES)),
                               trace=trace)
    full = np.empty((B, T, E), np.float32)
    for c in range(NCORES):
        b, hpar = c // 2, c % 2
        full[b].reshape(TK, 128, E)[hpar::2] = (
            res.results[c]["out"].reshape(NS, 128, E))
    return full, res


def kernel(**inputs) -> np.ndarray:
    out, _ = _run(inputs, trace=False)
    return out



# revision 21
# speedup vs baseline: 1.0543x; 1.0543x over previous
# Trainium2 Bass kernel for nn_DecoderBlock (B=4, T=2048, E=1024, H=16, D=64, FF=4096).
#
# Sharding: 8-way data parallel, zero collectives. Core c = 2*b + h handles batch b
# and the interleaved half of the sequence: 128-row q-blocks {2s+h : s=0..7}
# (1024 q rows per core). K/V are computed per-core for the full T=2048 rows of its
# batch (duplicated across the two cores of a batch pair) so attention needs no
# cross-core communication. The interleaved block assignment makes the causal
# work pattern identical on every core (uniform SPMD program): q-slot s statically
# attends keys [0, 256*(s+1)), with a per-core {0,1} multiplicative mask (input
# data) handling the parity-dependent diagonal.
#
# On-chip layout: activations flow feature-major (S^T = [k, q]) through attention so
# softmax needs no transposes of the probability matrix. Softmax uses no max
# subtraction (scores are ~N(0, 0.25^2) by construction); 1/denominator is
# exp(-ln(d)) so the whole kernel uses one ACT table set (exp/ln/relu).
# Matmuls run in bf16 with fp32 PSUM accumulation; LN statistics, residuals and
# the final output stay fp32. LN gains (g1,g2) and the attention 1/sqrt(E) scale
# are folded into the weights on the host; beta terms become per-feature biases.

import numpy as np
import ml_dtypes
from contextlib import ExitStack

BF16 = ml_dtypes.bfloat16

B, T, E, H, D, FF = 4, 2048, 1024, 16, 64, 4096
M = 1024          # q rows per core
NCORES = 8
NS = 8            # q slots (128 rows) per core
ET = E // 128     # 8 e-tiles
TK = T // 128     # 16 k-tiles
FT = FF // 128    # 32 ff-tiles
NP = H // 2       # 8 head pairs
EPS = 1e-5

_CACHE = {}


def _build(repeat=1):
    """Build (and cache) the Bass module for one core's uniform program.

    repeat>1 emits the whole body N times (identical I/O) — used only for
    slope-based wall-clock timing of one body on hardware.
    """
    key = ("nc", repeat)
    if key in _CACHE:
        return _CACHE[key]

    import concourse.bacc as bacc
    import concourse.tile as tile
    import concourse.mybir as mybir
    from concourse import masks as cmasks

    dt = mybir.dt
    f32, bf16 = dt.float32, dt.bfloat16
    AF = mybir.ActivationFunctionType
    OP = mybir.AluOpType

    nc = bacc.Bacc("TRN2", target_bir_lowering=False, debug=False,
                   num_devices=NCORES)

    # Every activation we use (Exp, Ln, Relu, Copy, Identity) lives in the
    # 'natural_log_exp_and_others' table set. The default per-function set
    # choice alternates home sets (exp_and_others vs natural_log), inserting
    # ~80 ACT table loads (~100us). Restrict the chooser to the one set that
    # covers everything -> a single load.
    import types
    import bass_rust as _br

    def _insert_act_loads_one_set(self):
        has_activation = any(
            isinstance(i, mybir.InstActivation)
            for b in self.main_func.blocks for i in b.instructions)
        if not has_activation:
            return
        tabs = bacc.get_activation_tables(self.m.arch)
        ours = {mybir.ActivationFunctionType.Exp, mybir.ActivationFunctionType.Ln,
                mybir.ActivationFunctionType.Relu, mybir.ActivationFunctionType.Copy,
                mybir.ActivationFunctionType.Identity}
        filt = []
        for name, fns in tabs.items():
            if name == "natural_log_exp_and_others":
                assert ours <= fns
                filt.append((name, fns))
            else:
                filt.append((name, fns - ours))
        _br.insert_act_table_loads(self, filt)

    nc.insert_act_table_loads = types.MethodType(_insert_act_loads_one_set, nc)

    # ----- DRAM I/O -----
    x_full = nc.dram_tensor("x_full", [T, E], f32, kind="ExternalInput").ap()
    xqp = nc.dram_tensor("xqp", [M, E], f32, kind="ExternalInput").ap()
    # weights arrive pre-arranged on the host into SBUF layout
    # [128 partitions, <tile dims>] so each DMA is one long contiguous run
    # per partition (minimal descriptor count).
    wq = nc.dram_tensor("wq", [128, ET * H * D], bf16, kind="ExternalInput").ap()
    wk = nc.dram_tensor("wk", [128, ET * H * D], bf16, kind="ExternalInput").ap()
    wv = nc.dram_tensor("wv", [128, ET * H * D], bf16, kind="ExternalInput").ap()
    projw = nc.dram_tensor("projw", [128, NP * E], bf16, kind="ExternalInput").ap()
    w1 = nc.dram_tensor("w1", [128, FT * ET * 128], bf16,
                        kind="ExternalInput").ap()
    w2 = nc.dram_tensor("w2", [128, FT * E], bf16, kind="ExternalInput").ap()
    qb_d = nc.dram_tensor("qb", [128, ET], f32, kind="ExternalInput").ap()
    kb_d = nc.dram_tensor("kb", [128, ET], f32, kind="ExternalInput").ap()
    vb_d = nc.dram_tensor("vb", [128, H * D], bf16, kind="ExternalInput").ap()
    b1_d = nc.dram_tensor("b1", [128, FT], f32, kind="ExternalInput").ap()
    bf2_d = nc.dram_tensor("bf2b", [128, E], bf16, kind="ExternalInput").ap()
    maskE_d = nc.dram_tensor("maskE", [128, 256], bf16, kind="ExternalInput").ap()
    maskO_d = nc.dram_tensor("maskO", [128, 256], bf16, kind="ExternalInput").ap()
    out = nc.dram_tensor("out", [M, E], f32, kind="ExternalOutput").ap()

    with tile.TileContext(nc) as tc:
      for _rep in range(repeat):
        es = ExitStack()
        with es:
            # ---------- constants (whole kernel) ----------
            constp = es.enter_context(tc.tile_pool(name="const", bufs=1))
            ident = constp.tile([128, 128], bf16)
            cmasks.make_identity(nc, ident[:])
            maskE = constp.tile([128, 256], bf16)
            nc.sync.dma_start(maskE[:], maskE_d)
            maskO = constp.tile([128, 256], bf16)
            nc.sync.dma_start(maskO[:], maskO_d)
            qb = constp.tile([128, ET], f32)
            nc.sync.dma_start(qb[:], qb_d)
            kb = constp.tile([128, ET], f32)
            nc.sync.dma_start(kb[:], kb_d)
            vb = constp.tile([128, H * D], bf16)
            nc.sync.dma_start(vb[:], vb_d)
            b1 = constp.tile([128, FT], f32)
            nc.sync.dma_start(b1[:], b1_d)
            bf2 = constp.tile([128, E], bf16)
            nc.sync.dma_start(bf2[:], bf2_d)
            eps_t = constp.tile([128, 1], f32)
            nc.gpsimd.memset(eps_t[:], EPS)

            # helper: layernorm one 128-row chunk (fp32 src slice in SBUF) and
            # write the transposed bf16 result into dst_T[:, et, col:col+128].
            def ln_stats(src, statp):
                st = statp.tile([128, 2, 6], f32, tag="st")
                for g in range(2):
                    nc.vector.bn_stats(st[:, g, :], src[:, g * 512:(g + 1) * 512])
                ag = statp.tile([128, 2], f32, tag="ag")
                nc.vector.bn_aggr(ag[:], st[:])
                lv = statp.tile([128, 1], f32, tag="lv")
                nc.scalar.activation(lv[:], ag[:, 1:2], AF.Ln, bias=eps_t[:])
                rstd = statp.tile([128, 1], f32, tag="rstd")
                nc.scalar.activation(rstd[:], lv[:], AF.Exp, scale=-0.5)
                return ag, rstd

            def ln_chunk(src, dst_T, col, statp, lnstage, tpsum, ci,
                         stats=None):
                ag, rstd = stats if stats is not None else ln_stats(src, statp)
                lc = lnstage.tile([128, E], bf16)
                # split the normalize-apply across DVE and GPSIMD so the
                # per-chunk chain latency halves and both engines share work
                nc.vector.tensor_scalar(lc[:, 0:512], src[:, 0:512],
                                        ag[:, 0:1], rstd[:],
                                        OP.subtract, OP.mult)
                nc.gpsimd.tensor_scalar(lc[:, 512:1024], src[:, 512:1024],
                                        ag[:, 0:1], rstd[:],
                                        OP.subtract, OP.mult)
                import os as _os
                if _os.environ.get("KT_DMA_TRANSPOSE", "0") == "1":
                    for et in range(ET):
                        nc.sync.dma_start(dst_T[:, et, col:col + 128],
                                            lc[:, et * 128:(et + 1) * 128],
                                            transpose=True)
                else:
                    for et in range(ET):
                        tp = tpsum.tile([128, 128], bf16)
                        nc.tensor.transpose(tp[:],
                                            lc[:, et * 128:(et + 1) * 128],
                                            ident[:])
                        dst = dst_T[:, et, col:col + 128]
                        if (et + ci) % 2 == 0:
                            nc.vector.tensor_copy(dst, tp[:])
                        else:
                            nc.scalar.copy(dst, tp[:])
                        # (copies stay off GPSIMD: it cannot read PSUM)

            # ---------- scope B: qT/kT/v (strict stack nesting) ----------
            xmid = es.enter_context(tc.tile_pool(name="xmidp", bufs=1)).tile(
                [128, NS, E], f32)
            with ExitStack() as sB:
                qT = sB.enter_context(tc.tile_pool(name="qTp", bufs=1)).tile(
                    [128, NP, M], bf16)
                kT = sB.enter_context(tc.tile_pool(name="kTp", bufs=1)).tile(
                    [128, NP, T], bf16)
                # per-pair AV stationary operand [V_h0 | ones(64) | V_h1]:
                # head0 reads cols 0:128, head1 cols 64:192 — the shared ones
                # block makes the same matmul that accumulates attn@V also
                # accumulate the softmax denominator (replicated on the 64
                # out-rows opposite each head's data rows).
                vS = sB.enter_context(tc.tile_pool(name="vp", bufs=1)).tile(
                    [128, TK, NP, 192], bf16)
                nc.gpsimd.memset(vS[:, :, :, 64:128], 1.0)

                # ---------- scope A: LN1 + QKV projections ----------
                with ExitStack() as sA:
                    wpool = sA.enter_context(tc.tile_pool(name="wpool", bufs=1))
                    stage = sA.enter_context(tc.tile_pool(name="xstage", bufs=3))
                    lnstage = sA.enter_context(tc.tile_pool(name="lnstage", bufs=3))
                    statp = sA.enter_context(tc.tile_pool(name="statp", bufs=6))
                    tpsum = sA.enter_context(
                        tc.tile_pool(name="tpsum", bufs=4, space="PSUM"))
                    qps = sA.enter_context(
                        tc.tile_pool(name="qps", bufs=2, space="PSUM"))

                    lnf = sA.enter_context(tc.tile_pool(name="lnfp", bufs=1)).tile(
                        [128, ET, T], bf16)
                    # LN1 over x_full (host-permuted: q rows are chunks 0..7,
                    # the pair-core's rows are chunks 8..15) -> lnf. Q^T only
                    # needs chunks 0..7, so it's emitted mid-loop.
                    wq_sb = wpool.tile([128, ET, H * D], bf16, tag="w")
                    nc.sync.dma_start(wq_sb[:], wq.rearrange(
                        "p (et n) -> p et n", et=ET))
                    for c in range(TK):
                        xc = stage.tile([128, E], f32)
                        nc.scalar.dma_start(
                            xc[:], x_full[c * 128:(c + 1) * 128, :])
                        ln_chunk(xc[:], lnf, c * 128, statp, lnstage,
                                 tpsum, c)
                        if c == NS - 1:
                            # Q^T = (wq)^T @ lnf[:, :, 0:M]   [hd, q]
                            for m in range(ET):
                                ps = qps.tile([128, 1024], f32)
                                for qc in range(2):
                                    for et in range(ET):
                                        nc.tensor.matmul(
                                            ps[:, qc * 512:(qc + 1) * 512],
                                            lhsT=wq_sb[:, et,
                                                       m * 128:(m + 1) * 128],
                                            rhs=lnf[:, et,
                                                    qc * 512:(qc + 1) * 512],
                                            start=(et == 0),
                                            stop=(et == ET - 1))
                                nc.vector.tensor_scalar_add(
                                    qT[:, m, :], ps[:], qb[:, m:m + 1])

                    # K^T [hd, t]
                    wk_sb = wpool.tile([128, ET, H * D], bf16, tag="w")
                    nc.sync.dma_start(wk_sb[:], wk.rearrange(
                        "p (et n) -> p et n", et=ET))
                    for m in range(ET):
                        for kh in range(2):
                            ps = qps.tile([128, 1024], f32)
                            for kc in range(2 * kh, 2 * kh + 2):
                                for et in range(ET):
                                    nc.tensor.matmul(
                                        ps[:, (kc % 2) * 512:(kc % 2 + 1) * 512],
                                        lhsT=wk_sb[:, et, m * 128:(m + 1) * 128],
                                        rhs=lnf[:, et, kc * 512:(kc + 1) * 512],
                                        start=(et == 0), stop=(et == ET - 1))
                            nc.vector.tensor_scalar_add(
                                kT[:, m, kh * 1024:(kh + 1) * 1024], ps[:],
                                kb[:, m:m + 1])

                    # V [t, hd] (token-major)
                    wv_sb = wpool.tile([128, ET, H * D], bf16, tag="w")
                    nc.sync.dma_start(wv_sb[:], wv.rearrange(
                        "p (et n) -> p et n", et=ET))
                    for t in range(TK):
                        ps = qps.tile([128, 1024], f32)
                        for hc in range(2):
                            for et in range(ET):
                                nc.tensor.matmul(
                                    ps[:, hc * 512:(hc + 1) * 512],
                                    lhsT=lnf[:, et, t * 128:(t + 1) * 128],
                                    rhs=wv_sb[:, et, hc * 512:(hc + 1) * 512],
                                    start=(et == 0), stop=(et == ET - 1))
                        psv = ps[:].rearrange("p (np h d) -> p np h d", np=NP,
                                              h=2)
                        vbv = vb[:].rearrange("p (np h d) -> p np h d", np=NP,
                                              h=2)
                        nc.vector.tensor_add(
                            vS[:, t, :, 0:64], psv[:, :, 0, :], vbv[:, :, 0, :])
                        nc.vector.tensor_add(
                            vS[:, t, :, 128:192], psv[:, :, 1, :],
                            vbv[:, :, 1, :])

                # ---------- attention + output projection ----------
                # half-outer: all pairs finish q-cols [0,512) first; the
                # output projection for q-chunks 0..3 is then dripped one
                # chunk at a time between second-half pairs so the PE works
                # through proj while ACT drains the exp/normalize backlog.
                with ExitStack() as sC:
                    oT = sC.enter_context(tc.tile_pool(name="oTp", bufs=1)).tile(
                        [128, NP, M], bf16)
                    ptp = sC.enter_context(tc.tile_pool(name="ptp", bufs=5))
                    normp = sC.enter_context(tc.tile_pool(name="normp", bufs=2))
                    xqpp = sC.enter_context(tc.tile_pool(name="xqpp", bufs=1))
                    pwp = sC.enter_context(tc.tile_pool(name="pwp", bufs=1))
                    apsum = sC.enter_context(
                        tc.tile_pool(name="apsum", bufs=2, space="PSUM"))
                    spsum = sC.enter_context(
                        tc.tile_pool(name="spsum", bufs=2, space="PSUM"))

                    maskEv = maskE[:].rearrange("p (h q) -> p h q", h=2)
                    maskOv = maskO[:].rearrange("p (h q) -> p h q", h=2)

                    pw_sb = pwp.tile([128, NP, E], bf16)
                    nc.sync.dma_start(pw_sb[:], projw.rearrange(
                        "p (m e) -> p m e", m=NP))
                    xq_t = {}

                    def do_norm(av, p, half):
                        # den sits on the 64 rows opposite each head's data;
                        # 1/den = exp(-ln(den)) stays partition-aligned, the
                        # multiply crosses bases (PSUM+SBUF operands).
                        tln = normp.tile([128, 2, 512], f32, tag="tln")
                        rcp = normp.tile([128, 2, 512], bf16, tag="rcp")
                        colr = slice(512 * half, 512 * (half + 1))
                        for h in range(2):
                            dn = slice(64 * (1 - h), 64 * (1 - h) + 64)
                            nc.scalar.activation(tln[dn, h, :],
                                                 av[dn, h, :], AF.Ln)
                            nc.scalar.activation(rcp[dn, h, :],
                                                 tln[dn, h, :], AF.Exp,
                                                 scale=-1.0)
                            nc.vector.tensor_mul(
                                oT[64 * h:64 * h + 64, p, colr],
                                av[64 * h:64 * h + 64, h, :],
                                rcp[dn, h, :])

                    # permuted key order: chunks 0..7 are this core's parity
                    # (incl. the causal diagonal), 8..15 the pair-core's
                    # (strictly past or future, selected by the all-ones/
                    # all-zero parity mask).
                    CHUNKS = [
                        [(kt, 128 * kt, "tri") for kt in range(4)] +
                        [(kt, 128 * (kt - 8), "par") for kt in range(8, 12)],
                        [(kt, 0, None) for kt in range(4)] +
                        [(kt, 128 * (kt - 4), "tri") for kt in range(4, 8)] +
                        [(kt, 0, None) for kt in range(8, 12)] +
                        [(kt, 128 * (kt - 12), "par") for kt in range(12, 16)],
                    ]

                    def do_av(item):
                        av, p, half, kt, qlo, sp, pt = item
                        for h in range(2):
                            nc.tensor.matmul(
                                av[:, h, qlo:512],
                                lhsT=vS[:, kt, p, 64 * h:64 * h + 128],
                                rhs=pt[:, h, qlo:512],
                                start=(kt == 0), stop=sp,
                                skip_group_check=True)
                        if sp:
                            do_norm(av, p, half)

                    def proj_qm(qm):
                        if qm // 4 not in xq_t:
                            xqh = xqpp.tile([128, 4, E], f32, tag="xq")
                            xq_t[qm // 4] = xqh
                        xqh = xq_t[qm // 4]
                        nc.sync.dma_start(
                            xqh[:, qm % 4, :], xqp[qm * 128:(qm + 1) * 128, :])
                        ps = apsum.tile([128, 1024], f32, tag="av")
                        for ec in range(2):
                            for pk in range(NP):
                                nc.tensor.matmul(
                                    ps[:, ec * 512:(ec + 1) * 512],
                                    lhsT=oT[:, pk, qm * 128:(qm + 1) * 128],
                                    rhs=pw_sb[:, pk, ec * 512:(ec + 1) * 512],
                                    start=(pk == 0), stop=(pk == NP - 1))
                        nc.vector.tensor_add(
                            xmid[:, qm, :], ps[:], xqh[:, qm % 4, :])

                    def attn_half(half, interleave=()):
                        chunks = CHUNKS[half]
                        last_kt = chunks[-1][0]
                        pend = []
                        for p in range(NP):
                            av = apsum.tile([128, 2, 512], f32, tag="av")
                            for kt, qlo, mk in chunks:
                                ps = spsum.tile([128, 2, 512], f32)
                                for h in range(2):
                                    nc.tensor.matmul(
                                        ps[:, h, qlo:512],
                                        lhsT=kT[64 * h:64 * h + 64, p,
                                                kt * 128:(kt + 1) * 128],
                                        rhs=qT[64 * h:64 * h + 64, p,
                                               512 * half + qlo:
                                               512 * (half + 1)],
                                        start=True, stop=True)
                                pt = ptp.tile([128, 2, 512], bf16)
                                nc.scalar.activation(
                                    pt[:, :, qlo:512], ps[:, :, qlo:512],
                                    AF.Exp)
                                if mk is not None:
                                    mkv = maskEv if mk == "tri" else maskOv
                                    nc.vector.tensor_mul(
                                        pt[:, :, qlo:qlo + 128],
                                        pt[:, :, qlo:qlo + 128], mkv)
                                pend.append((av, p, half, kt, qlo,
                                             kt == last_kt, pt))
                                if len(pend) > 3:
                                    do_av(pend.pop(0))
                            if p % 2 == 1 and p // 2 < len(interleave):
                                while pend:
                                    do_av(pend.pop(0))
                                interleave[p // 2]()
                        while pend:
                            do_av(pend.pop(0))

                    attn_half(0)
                    attn_half(1, interleave=[
                        (lambda qm=qm: proj_qm(qm)) for qm in range(4)])
                    for qm in range(4, NS):
                        proj_qm(qm)

            # ---------- scope D: LN2 + FFN ----------
            # LN2 q-chunks 0..3 interleave with the tail projections; FFN1's
            # first-half fm groups interleave with LN2 chunks 4..7 so the
            # PE never drains while DVE/ACT run the layernorm chains.
            with ExitStack() as sD:
                ln2T = sD.enter_context(tc.tile_pool(name="ln2p", bufs=1)).tile(
                    [128, ET, M], bf16)
                w2_sb = sD.enter_context(tc.tile_pool(name="w2p", bufs=1)).tile(
                    [128, FT, E], bf16)
                nc.sync.dma_start(w2_sb[:], w2.rearrange(
                    "p (ft e) -> p ft e", ft=FT))

                statp2 = sD.enter_context(tc.tile_pool(name="statp2", bufs=6))
                lnstage2 = sD.enter_context(
                    tc.tile_pool(name="lnstage2", bufs=3))
                tpsum2 = sD.enter_context(
                    tc.tile_pool(name="tpsum2", bufs=2, space="PSUM"))
                rtp = sD.enter_context(tc.tile_pool(name="rtp", bufs=1))
                w1p = sD.enter_context(tc.tile_pool(name="w1p", bufs=4))
                zps = sD.enter_context(
                    tc.tile_pool(name="zps", bufs=2, space="PSUM"))
                ops = sD.enter_context(
                    tc.tile_pool(name="ops", bufs=2, space="PSUM"))
                outp = sD.enter_context(tc.tile_pool(name="outp", bufs=3))

                def ln2_qm(qm):
                    ln_chunk(xmid[:, qm, :], ln2T, qm * 128, statp2,
                             lnstage2, tpsum2, qm)
                    # after LN2 consumed xmid, fold the final bf2 bias in
                    nc.vector.tensor_add(xmid[:, qm, :], xmid[:, qm, :],
                                         bf2[:])

                rT_tiles = {}

                def ffn1_fm(half, fm):
                    if half not in rT_tiles:
                        rT = rtp.tile([128, FT, 512], bf16, tag="rT")
                        rT_tiles[half] = rT
                    rT = rT_tiles[half]
                    w1f = w1p.tile([128, ET, 128], bf16)
                    nc.sync.dma_start(
                        w1f[:], w1.rearrange("p (fm et f) -> p fm et f",
                                             fm=FT, et=ET)[:, fm])
                    zp = zps.tile([128, 512], f32)
                    for et in range(ET):
                        nc.tensor.matmul(
                            zp[:],
                            lhsT=w1f[:, et, :],
                            rhs=ln2T[:, et, half * 512:(half + 1) * 512],
                            start=(et == 0), stop=(et == ET - 1))
                    nc.scalar.activation(rT[:, fm, :], zp[:], AF.Relu,
                                         bias=b1[:, fm:fm + 1])

                def ffn2_qq(half, qq):
                    rT = rT_tiles[half]
                    qm = half * 4 + qq
                    ot = outp.tile([128, E], f32)
                    op = ops.tile([128, 1024], f32)
                    for ec in range(2):
                        for fk in range(FT):
                            nc.tensor.matmul(
                                op[:, ec * 512:(ec + 1) * 512],
                                lhsT=rT[:, fk, qq * 128:(qq + 1) * 128],
                                rhs=w2_sb[:, fk, ec * 512:(ec + 1) * 512],
                                start=(fk == 0), stop=(fk == FT - 1))
                    nc.vector.tensor_add(ot[:], op[:], xmid[:, qm, :])
                    nc.scalar.dma_start(out[qm * 128:(qm + 1) * 128, :], ot[:])

                for qm in range(4):
                    ln2_qm(qm)
                # LN2 chunks 4..7 drip between the first 4 fm-groups of
                # FFN1-half0 (which only needs ln2T token cols 0:512).
                for g in range(4):
                    ln2_qm(4 + g)
                    for fm in range(g * 8, g * 8 + 8):
                        ffn1_fm(0, fm)
                for qq in range(4):
                    ffn2_qq(0, qq)
                rT_tiles.pop(0)
                for fm in range(FT):
                    ffn1_fm(1, fm)
                for qq in range(4):
                    ffn2_qq(1, qq)

    nc.compile()
    _CACHE[key] = nc
    return nc


def _prep_inputs(x, wq, wk, wv, proj_w, proj_b, g1, beta1, g2, beta2, w1, bf1,
                 w2, bf2):
    """Host-side sharding + weight folding. Returns list of 8 in_maps."""
    f32 = np.float32
    x = np.asarray(x, f32)
    scale = float(E) ** -0.5

    Wq = np.asarray(wq, f32).transpose(1, 0, 2).reshape(E, H * D) * scale
    Wk = np.asarray(wk, f32).transpose(1, 0, 2).reshape(E, H * D)
    Wv = np.asarray(wv, f32).transpose(1, 0, 2).reshape(E, H * D)
    g1 = np.asarray(g1, f32)
    beta1 = np.asarray(beta1, f32)
    g2 = np.asarray(g2, f32)
    beta2 = np.asarray(beta2, f32)
    w1 = np.asarray(w1, f32)
    w2 = np.asarray(w2, f32)
    bf1 = np.asarray(bf1, f32)
    bf2 = np.asarray(bf2, f32)
    proj_w = np.asarray(proj_w, f32)
    proj_b = np.asarray(proj_b, f32)

    def sb_layout(w, ntile):
        # [ntile*128, N] -> [128, ntile*N] with per-partition contiguous tiles
        n = w.shape[1]
        return np.ascontiguousarray(
            w.reshape(ntile, 128, n).transpose(1, 0, 2).reshape(128, ntile * n))

    wq_b = sb_layout((Wq * g1[:, None]).astype(BF16), ET)
    wk_b = sb_layout((Wk * g1[:, None]).astype(BF16), ET)
    wv_b = sb_layout((Wv * g1[:, None]).astype(BF16), ET)
    qbias = beta1 @ Wq
    kbias = beta1 @ Wk
    vbias = beta1 @ Wv
    w1_b = np.ascontiguousarray(
        (w1 * g2[:, None]).astype(BF16)
        .reshape(ET, 128, FT, 128).transpose(1, 2, 0, 3)
        .reshape(128, FT * ET * 128))
    b1v = bf1 + beta2 @ w1
    w2_b = sb_layout(w2.astype(BF16), FT)
    projw_b = sb_layout(proj_w.astype(BF16), NP)

    qb = np.ascontiguousarray(qbias.reshape(ET, 128).T, f32)
    kb = np.ascontiguousarray(kbias.reshape(ET, 128).T, f32)
    vb = np.ascontiguousarray(np.broadcast_to(vbias, (128, H * D))).astype(BF16)
    b1m = np.ascontiguousarray(b1v.reshape(FT, 128).T, f32)
    bf2m = np.ascontiguousarray(np.broadcast_to(bf2, (128, E))).astype(BF16)

    tri = np.triu(np.ones((128, 128), f32))  # [k_row, q_col]: 1 iff k <= q
    onesm = np.ones((128, 128), f32)
    zerosm = np.zeros((128, 128), f32)
    # maskE = causal diagonal (all cores); maskO = parity: the pair-core's
    # diagonal-adjacent chunk is strictly past (odd cores) or future (even).
    mO = {0: zerosm, 1: tri * 0 + 1}

    in_maps = []
    for c in range(NCORES):
        b, hpar = c // 2, c % 2
        xc = x[b].reshape(TK, 128, E)
        xq = np.ascontiguousarray(xc[hpar::2].reshape(M, E), f32)
        xperm = np.ascontiguousarray(
            np.concatenate([xc[hpar::2], xc[1 - hpar::2]], axis=0)
            .reshape(T, E), f32)
        in_maps.append({
            "x_full": xperm,
            "xqp": xq + proj_b[None, :].astype(f32),
            "wq": wq_b, "wk": wk_b, "wv": wv_b,
            "projw": projw_b, "w1": w1_b, "w2": w2_b,
            "qb": qb, "kb": kb, "vb": vb, "b1": b1m, "bf2b": bf2m,
            "maskE": np.ascontiguousarray(
                np.tile(tri, (1, 2))).astype(BF16),
            "maskO": np.ascontiguousarray(
                np.tile(mO[hpar], (1, 2))).astype(BF16),
        })
    return in_maps


def _run(inputs, trace=False):
    from concourse.bass_utils import run_bass_kernel_spmd
    nc = _build()
    in_maps = _prep_inputs(**inputs)
    res = run_bass_kernel_spmd(nc, in_maps, core_ids=list(range(NCORES)),
                               trace=trace)
    full = np.empty((B, T, E), np.float32)
    for c in range(NCORES):
        b, hpar = c // 2, c % 2
        full[b].reshape(TK, 128, E)[hpar::2] = (
            res.results[c]["out"].reshape(NS, 128, E))
    return full, res


def kernel(**inputs) -> np.ndarray:
    out, _ = _run(inputs, trace=False)
    return out



# revision 24
# speedup vs baseline: 1.0757x; 1.0203x over previous
# Trainium2 Bass kernel for nn_DecoderBlock (B=4, T=2048, E=1024, H=16, D=64, FF=4096).
#
# Sharding: 8-way data parallel, zero collectives. Core c = 2*b + h handles batch b
# and the interleaved half of the sequence: 128-row q-blocks {2s+h : s=0..7}
# (1024 q rows per core). K/V are computed per-core for the full T=2048 rows of its
# batch (duplicated across the two cores of a batch pair) so attention needs no
# cross-core communication. The interleaved block assignment makes the causal
# work pattern identical on every core (uniform SPMD program): q-slot s statically
# attends keys [0, 256*(s+1)), with a per-core {0,1} multiplicative mask (input
# data) handling the parity-dependent diagonal.
#
# On-chip layout: activations flow feature-major (S^T = [k, q]) through attention so
# softmax needs no transposes of the probability matrix. Softmax uses no max
# subtraction (scores are ~N(0, 0.25^2) by construction); 1/denominator is
# exp(-ln(d)) so the whole kernel uses one ACT table set (exp/ln/relu).
# Matmuls run in bf16 with fp32 PSUM accumulation; LN statistics, residuals and
# the final output stay fp32. LN gains (g1,g2) and the attention 1/sqrt(E) scale
# are folded into the weights on the host; beta terms become per-feature biases.

import numpy as np
import ml_dtypes
from contextlib import ExitStack

BF16 = ml_dtypes.bfloat16

B, T, E, H, D, FF = 4, 2048, 1024, 16, 64, 4096
M = 1024          # q rows per core
NCORES = 8
NS = 8            # q slots (128 rows) per core
ET = E // 128     # 8 e-tiles
TK = T // 128     # 16 k-tiles
FT = FF // 128    # 32 ff-tiles
NP = H // 2       # 8 head pairs
EPS = 1e-5

_CACHE = {}


def _build(repeat=1):
    """Build (and cache) the Bass module for one core's uniform program.

    repeat>1 emits the whole body N times (identical I/O) — used only for
    slope-based wall-clock timing of one body on hardware.
    """
    key = ("nc", repeat)
    if key in _CACHE:
        return _CACHE[key]

    import concourse.bacc as bacc
    import concourse.tile as tile
    import concourse.mybir as mybir
    from concourse import masks as cmasks

    dt = mybir.dt
    f32, bf16 = dt.float32, dt.bfloat16
    AF = mybir.ActivationFunctionType
    OP = mybir.AluOpType

    nc = bacc.Bacc("TRN2", target_bir_lowering=False, debug=False,
                   num_devices=NCORES)

    # Every activation we use (Exp, Ln, Relu, Copy, Identity) lives in the
    # 'natural_log_exp_and_others' table set. The default per-function set
    # choice alternates home sets (exp_and_others vs natural_log), inserting
    # ~80 ACT table loads (~100us). Restrict the chooser to the one set that
    # covers everything -> a single load.
    import types
    import bass_rust as _br

    def _insert_act_loads_one_set(self):
        has_activation = any(
            isinstance(i, mybir.InstActivation)
            for b in self.main_func.blocks for i in b.instructions)
        if not has_activation:
            return
        tabs = bacc.get_activation_tables(self.m.arch)
        ours = {mybir.ActivationFunctionType.Exp, mybir.ActivationFunctionType.Ln,
                mybir.ActivationFunctionType.Relu, mybir.ActivationFunctionType.Copy,
                mybir.ActivationFunctionType.Identity}
        filt = []
        for name, fns in tabs.items():
            if name == "natural_log_exp_and_others":
                assert ours <= fns
                filt.append((name, fns))
            else:
                filt.append((name, fns - ours))
        _br.insert_act_table_loads(self, filt)

    nc.insert_act_table_loads = types.MethodType(_insert_act_loads_one_set, nc)

    # ----- DRAM I/O -----
    x_full = nc.dram_tensor("x_full", [T, E], bf16, kind="ExternalInput").ap()
    xqp = nc.dram_tensor("xqp", [M, E], f32, kind="ExternalInput").ap()
    # weights arrive pre-arranged on the host into SBUF layout
    # [128 partitions, <tile dims>] so each DMA is one long contiguous run
    # per partition (minimal descriptor count).
    wq = nc.dram_tensor("wq", [128, ET * H * D], bf16, kind="ExternalInput").ap()
    wk = nc.dram_tensor("wk", [128, ET * H * D], bf16, kind="ExternalInput").ap()
    wv = nc.dram_tensor("wv", [128, ET * H * D], bf16, kind="ExternalInput").ap()
    projw = nc.dram_tensor("projw", [128, NP * E], bf16, kind="ExternalInput").ap()
    w1 = nc.dram_tensor("w1", [128, FT * ET * 128], bf16,
                        kind="ExternalInput").ap()
    w2 = nc.dram_tensor("w2", [128, FT * E], bf16, kind="ExternalInput").ap()
    qb_d = nc.dram_tensor("qb", [128, ET], f32, kind="ExternalInput").ap()
    kb_d = nc.dram_tensor("kb", [128, ET], f32, kind="ExternalInput").ap()
    vb_d = nc.dram_tensor("vb", [128, H * D], bf16, kind="ExternalInput").ap()
    b1_d = nc.dram_tensor("b1", [128, FT], f32, kind="ExternalInput").ap()
    bf2_d = nc.dram_tensor("bf2b", [128, E], bf16, kind="ExternalInput").ap()
    maskE_d = nc.dram_tensor("maskE", [128, 256], bf16, kind="ExternalInput").ap()
    maskO_d = nc.dram_tensor("maskO", [128, 256], bf16, kind="ExternalInput").ap()
    out = nc.dram_tensor("out", [M, E], f32, kind="ExternalOutput").ap()

    with tile.TileContext(nc) as tc:
      for _rep in range(repeat):
        es = ExitStack()
        with es:
            # ---------- constants (whole kernel) ----------
            constp = es.enter_context(tc.tile_pool(name="const", bufs=1))
            ident = constp.tile([128, 128], bf16)
            cmasks.make_identity(nc, ident[:])
            maskE = constp.tile([128, 256], bf16)
            nc.sync.dma_start(maskE[:], maskE_d)
            maskO = constp.tile([128, 256], bf16)
            nc.sync.dma_start(maskO[:], maskO_d)
            qb = constp.tile([128, ET], f32)
            nc.sync.dma_start(qb[:], qb_d)
            kb = constp.tile([128, ET], f32)
            nc.sync.dma_start(kb[:], kb_d)
            vb = constp.tile([128, H * D], bf16)
            nc.sync.dma_start(vb[:], vb_d)
            b1 = constp.tile([128, FT], f32)
            nc.sync.dma_start(b1[:], b1_d)
            bf2 = constp.tile([128, E], bf16)
            nc.sync.dma_start(bf2[:], bf2_d)
            eps_t = constp.tile([128, 1], f32)
            nc.gpsimd.memset(eps_t[:], EPS)

            # helper: layernorm one 128-row chunk (fp32 src slice in SBUF) and
            # write the transposed bf16 result into dst_T[:, et, col:col+128].
            def ln_stats(src, statp):
                st = statp.tile([128, 2, 6], f32, tag="st")
                for g in range(2):
                    nc.vector.bn_stats(st[:, g, :], src[:, g * 512:(g + 1) * 512])
                ag = statp.tile([128, 2], f32, tag="ag")
                nc.vector.bn_aggr(ag[:], st[:])
                lv = statp.tile([128, 1], f32, tag="lv")
                nc.scalar.activation(lv[:], ag[:, 1:2], AF.Ln, bias=eps_t[:])
                rstd = statp.tile([128, 1], f32, tag="rstd")
                nc.scalar.activation(rstd[:], lv[:], AF.Exp, scale=-0.5)
                return ag, rstd

            def ln_chunk(src, dst_T, col, statp, lnstage, tpsum, ci,
                         stats=None):
                ag, rstd = stats if stats is not None else ln_stats(src, statp)
                lc = lnstage.tile([128, E], bf16)
                # split the normalize-apply across DVE and GPSIMD so the
                # per-chunk chain latency halves and both engines share work
                nc.vector.tensor_scalar(lc[:, 0:512], src[:, 0:512],
                                        ag[:, 0:1], rstd[:],
                                        OP.subtract, OP.mult)
                nc.gpsimd.tensor_scalar(lc[:, 512:1024], src[:, 512:1024],
                                        ag[:, 0:1], rstd[:],
                                        OP.subtract, OP.mult)
                import os as _os
                if _os.environ.get("KT_DMA_TRANSPOSE", "0") == "1":
                    for et in range(ET):
                        nc.sync.dma_start(dst_T[:, et, col:col + 128],
                                            lc[:, et * 128:(et + 1) * 128],
                                            transpose=True)
                else:
                    for et in range(ET):
                        tp = tpsum.tile([128, 128], bf16)
                        nc.tensor.transpose(tp[:],
                                            lc[:, et * 128:(et + 1) * 128],
                                            ident[:])
                        dst = dst_T[:, et, col:col + 128]
                        if (et + ci) % 2 == 0:
                            nc.vector.tensor_copy(dst, tp[:])
                        else:
                            nc.scalar.copy(dst, tp[:])
                        # (copies stay off GPSIMD: it cannot read PSUM)

            # ---------- scope B: qT/kT/v (strict stack nesting) ----------
            xmid = es.enter_context(tc.tile_pool(name="xmidp", bufs=1)).tile(
                [128, NS, E], bf16)
            with ExitStack() as sB:
                qT = sB.enter_context(tc.tile_pool(name="qTp", bufs=1)).tile(
                    [128, NP, M], bf16)
                kT = sB.enter_context(tc.tile_pool(name="kTp", bufs=1)).tile(
                    [128, NP, T], bf16)
                # per-pair AV stationary operand [V_h0 | ones(64) | V_h1]:
                # head0 reads cols 0:128, head1 cols 64:192 — the shared ones
                # block makes the same matmul that accumulates attn@V also
                # accumulate the softmax denominator (replicated on the 64
                # out-rows opposite each head's data rows).
                vS = sB.enter_context(tc.tile_pool(name="vp", bufs=1)).tile(
                    [128, TK, NP, 192], bf16)
                nc.gpsimd.memset(vS[:, :, :, 64:128], 1.0)

                # ---------- scope A: LN1 + QKV projections ----------
                with ExitStack() as sA:
                    wpool = sA.enter_context(tc.tile_pool(name="wpool", bufs=1))
                    stage = sA.enter_context(tc.tile_pool(name="xstage", bufs=2))
                    lnstage = sA.enter_context(tc.tile_pool(name="lnstage", bufs=2))
                    statp = sA.enter_context(tc.tile_pool(name="statp", bufs=6))
                    tpsum = sA.enter_context(
                        tc.tile_pool(name="tpsum", bufs=4, space="PSUM"))
                    qps = sA.enter_context(
                        tc.tile_pool(name="qps", bufs=2, space="PSUM"))

                    lnf = sA.enter_context(tc.tile_pool(name="lnfp", bufs=1)).tile(
                        [128, ET, T], bf16)
                    # LN1 over x_full (host-permuted: q rows are chunks 0..7,
                    # the pair-core's rows are chunks 8..15) -> lnf. Q^T, each
                    # K^T token-quarter and each V chunk are emitted as soon
                    # as the lnf region they read is complete, so the PE
                    # works through projections while the LN chains run.
                    wq_sb = wpool.tile([128, ET, H * D], bf16, tag="wq")
                    nc.sync.dma_start(wq_sb[:], wq.rearrange(
                        "p (et n) -> p et n", et=ET))
                    wk_sb = wpool.tile([128, ET, H * D], bf16, tag="wk")
                    nc.sync.dma_start(wk_sb[:], wk.rearrange(
                        "p (et n) -> p et n", et=ET))
                    wv_sb = wpool.tile([128, ET, H * D], bf16, tag="wv")
                    nc.sync.dma_start(wv_sb[:], wv.rearrange(
                        "p (et n) -> p et n", et=ET))

                    def v_chunk(t):
                        ps = qps.tile([128, 1024], f32)
                        for hc in range(2):
                            for et in range(ET):
                                nc.tensor.matmul(
                                    ps[:, hc * 512:(hc + 1) * 512],
                                    lhsT=lnf[:, et, t * 128:(t + 1) * 128],
                                    rhs=wv_sb[:, et, hc * 512:(hc + 1) * 512],
                                    start=(et == 0), stop=(et == ET - 1))
                        psv = ps[:].rearrange("p (np h d) -> p np h d", np=NP,
                                              h=2)
                        vbv = vb[:].rearrange("p (np h d) -> p np h d", np=NP,
                                              h=2)
                        nc.vector.tensor_add(
                            vS[:, t, :, 0:64], psv[:, :, 0, :], vbv[:, :, 0, :])
                        nc.vector.tensor_add(
                            vS[:, t, :, 128:192], psv[:, :, 1, :],
                            vbv[:, :, 1, :])

                    def k_quarter(tq):
                        for m in range(ET):
                            ps = qps.tile([128, 512], f32)
                            for et in range(ET):
                                nc.tensor.matmul(
                                    ps[:],
                                    lhsT=wk_sb[:, et, m * 128:(m + 1) * 128],
                                    rhs=lnf[:, et, tq * 512:(tq + 1) * 512],
                                    start=(et == 0), stop=(et == ET - 1))
                            nc.vector.tensor_scalar_add(
                                kT[:, m, tq * 512:(tq + 1) * 512], ps[:],
                                kb[:, m:m + 1])

                    def q_proj():
                        for m in range(ET):
                            ps = qps.tile([128, 1024], f32)
                            for qc in range(2):
                                for et in range(ET):
                                    nc.tensor.matmul(
                                        ps[:, qc * 512:(qc + 1) * 512],
                                        lhsT=wq_sb[:, et,
                                                   m * 128:(m + 1) * 128],
                                        rhs=lnf[:, et,
                                                qc * 512:(qc + 1) * 512],
                                        start=(et == 0), stop=(et == ET - 1))
                            nc.vector.tensor_scalar_add(
                                qT[:, m, :], ps[:], qb[:, m:m + 1])

                    for c in range(TK):
                        xc = stage.tile([128, E], bf16)
                        nc.scalar.dma_start(
                            xc[:], x_full[c * 128:(c + 1) * 128, :])
                        ln_chunk(xc[:], lnf, c * 128, statp, lnstage,
                                 tpsum, c)
                        v_chunk(c)
                        if c % 4 == 3:
                            k_quarter(c // 4)
                        if c == NS - 1:
                            q_proj()

                # ---------- attention + output projection ----------
                # half-outer: all pairs finish q-cols [0,512) first; the
                # output projection for q-chunks 0..3 is then dripped one
                # chunk at a time between second-half pairs so the PE works
                # through proj while ACT drains the exp/normalize backlog.
                with ExitStack() as sC:
                    oT = sC.enter_context(tc.tile_pool(name="oTp", bufs=1)).tile(
                        [128, NP, M], bf16)
                    ptp = sC.enter_context(tc.tile_pool(name="ptp", bufs=5))
                    normp = sC.enter_context(tc.tile_pool(name="normp", bufs=2))
                    xqpp = sC.enter_context(tc.tile_pool(name="xqpp", bufs=1))
                    pwp = sC.enter_context(tc.tile_pool(name="pwp", bufs=1))
                    apsum = sC.enter_context(
                        tc.tile_pool(name="apsum", bufs=2, space="PSUM"))
                    spsum = sC.enter_context(
                        tc.tile_pool(name="spsum", bufs=2, space="PSUM"))

                    maskEv = maskE[:].rearrange("p (h q) -> p h q", h=2)
                    maskOv = maskO[:].rearrange("p (h q) -> p h q", h=2)

                    pw_sb = pwp.tile([128, NP, E], bf16)
                    nc.sync.dma_start(pw_sb[:], projw.rearrange(
                        "p (m e) -> p m e", m=NP))
                    xq_t = {}

                    def do_norm(av, p, half):
                        # den sits on the 64 rows opposite each head's data;
                        # 1/den = exp(-ln(den)) stays partition-aligned, the
                        # multiply crosses bases (PSUM+SBUF operands).
                        tln = normp.tile([128, 2, 512], f32, tag="tln")
                        rcp = normp.tile([128, 2, 512], bf16, tag="rcp")
                        colr = slice(512 * half, 512 * (half + 1))
                        for h in range(2):
                            dn = slice(64 * (1 - h), 64 * (1 - h) + 64)
                            nc.scalar.activation(tln[dn, h, :],
                                                 av[dn, h, :], AF.Ln)
                            nc.scalar.activation(rcp[dn, h, :],
                                                 tln[dn, h, :], AF.Exp,
                                                 scale=-1.0)
                            nc.vector.tensor_mul(
                                oT[64 * h:64 * h + 64, p, colr],
                                av[64 * h:64 * h + 64, h, :],
                                rcp[dn, h, :])

                    # permuted key order: chunks 0..7 are this core's parity
                    # (incl. the causal diagonal), 8..15 the pair-core's
                    # (strictly past or future, selected by the all-ones/
                    # all-zero parity mask).
                    CHUNKS = [
                        [(kt, 128 * kt, "tri") for kt in range(4)] +
                        [(kt, 128 * (kt - 8), "par") for kt in range(8, 12)],
                        [(kt, 0, None) for kt in range(4)] +
                        [(kt, 128 * (kt - 4), "tri") for kt in range(4, 8)] +
                        [(kt, 0, None) for kt in range(8, 12)] +
                        [(kt, 128 * (kt - 12), "par") for kt in range(12, 16)],
                    ]

                    def do_av(item):
                        av, p, half, kt, qlo, sp, pt = item
                        for h in range(2):
                            nc.tensor.matmul(
                                av[:, h, qlo:512],
                                lhsT=vS[:, kt, p, 64 * h:64 * h + 128],
                                rhs=pt[:, h, qlo:512],
                                start=(kt == 0), stop=sp,
                                skip_group_check=True)
                        if sp:
                            do_norm(av, p, half)

                    def proj_qm(qm):
                        if qm // 4 not in xq_t:
                            xqh = xqpp.tile([128, 4, E], f32, tag="xq")
                            xq_t[qm // 4] = xqh
                        xqh = xq_t[qm // 4]
                        nc.sync.dma_start(
                            xqh[:, qm % 4, :], xqp[qm * 128:(qm + 1) * 128, :])
                        ps = apsum.tile([128, 1024], f32, tag="av")
                        for ec in range(2):
                            for pk in range(NP):
                                nc.tensor.matmul(
                                    ps[:, ec * 512:(ec + 1) * 512],
                                    lhsT=oT[:, pk, qm * 128:(qm + 1) * 128],
                                    rhs=pw_sb[:, pk, ec * 512:(ec + 1) * 512],
                                    start=(pk == 0), stop=(pk == NP - 1))
                        nc.vector.tensor_add(
                            xmid[:, qm, :], ps[:], xqh[:, qm % 4, :])

                    def attn_half(half, interleave=()):
                        chunks = CHUNKS[half]
                        last_kt = chunks[-1][0]
                        pend = []
                        for p in range(NP):
                            av = apsum.tile([128, 2, 512], f32, tag="av")
                            for kt, qlo, mk in chunks:
                                ps = spsum.tile([128, 2, 512], f32)
                                for h in range(2):
                                    nc.tensor.matmul(
                                        ps[:, h, qlo:512],
                                        lhsT=kT[64 * h:64 * h + 64, p,
                                                kt * 128:(kt + 1) * 128],
                                        rhs=qT[64 * h:64 * h + 64, p,
                                               512 * half + qlo:
                                               512 * (half + 1)],
                                        start=True, stop=True)
                                pt = ptp.tile([128, 2, 512], bf16)
                                nc.scalar.activation(
                                    pt[:, :, qlo:512], ps[:, :, qlo:512],
                                    AF.Exp)
                                if mk is not None:
                                    mkv = maskEv if mk == "tri" else maskOv
                                    nc.vector.tensor_mul(
                                        pt[:, :, qlo:qlo + 128],
                                        pt[:, :, qlo:qlo + 128], mkv)
                                pend.append((av, p, half, kt, qlo,
                                             kt == last_kt, pt))
                                if len(pend) > 3:
                                    do_av(pend.pop(0))
                            if p % 2 == 1 and p // 2 < len(interleave):
                                while pend:
                                    do_av(pend.pop(0))
                                interleave[p // 2]()
                        while pend:
                            do_av(pend.pop(0))

                    attn_half(0)
                    attn_half(1, interleave=[
                        (lambda qm=qm: proj_qm(qm)) for qm in range(4)])
                    for qm in range(4, NS):
                        proj_qm(qm)

            # ---------- scope D: LN2 + FFN ----------
            # LN2 q-chunks 0..3 interleave with the tail projections; FFN1's
            # first-half fm groups interleave with LN2 chunks 4..7 so the
            # PE never drains while DVE/ACT run the layernorm chains.
            with ExitStack() as sD:
                ln2T = sD.enter_context(tc.tile_pool(name="ln2p", bufs=1)).tile(
                    [128, ET, M], bf16)
                w2_sb = sD.enter_context(tc.tile_pool(name="w2p", bufs=1)).tile(
                    [128, FT, E], bf16)
                nc.sync.dma_start(w2_sb[:], w2.rearrange(
                    "p (ft e) -> p ft e", ft=FT))

                statp2 = sD.enter_context(tc.tile_pool(name="statp2", bufs=6))
                lnstage2 = sD.enter_context(
                    tc.tile_pool(name="lnstage2", bufs=3))
                tpsum2 = sD.enter_context(
                    tc.tile_pool(name="tpsum2", bufs=2, space="PSUM"))
                rtp = sD.enter_context(tc.tile_pool(name="rtp", bufs=1))
                w1p = sD.enter_context(tc.tile_pool(name="w1p", bufs=4))
                zps = sD.enter_context(
                    tc.tile_pool(name="zps", bufs=2, space="PSUM"))
                ops = sD.enter_context(
                    tc.tile_pool(name="ops", bufs=2, space="PSUM"))
                outp = sD.enter_context(tc.tile_pool(name="outp", bufs=3))

                def ln2_qm(qm):
                    ln_chunk(xmid[:, qm, :], ln2T, qm * 128, statp2,
                             lnstage2, tpsum2, qm)
                    # after LN2 consumed xmid, fold the final bf2 bias in
                    nc.vector.tensor_add(xmid[:, qm, :], xmid[:, qm, :],
                                         bf2[:])

                rT_tiles = {}

                def ffn1_fm(half, fm):
                    if half not in rT_tiles:
                        rT = rtp.tile([128, FT, 512], bf16, tag="rT")
                        rT_tiles[half] = rT
                    rT = rT_tiles[half]
                    w1f = w1p.tile([128, ET, 128], bf16)
                    nc.sync.dma_start(
                        w1f[:], w1.rearrange("p (fm et f) -> p fm et f",
                                             fm=FT, et=ET)[:, fm])
                    zp = zps.tile([128, 512], f32)
                    for et in range(ET):
                        nc.tensor.matmul(
                            zp[:],
                            lhsT=w1f[:, et, :],
                            rhs=ln2T[:, et, half * 512:(half + 1) * 512],
                            start=(et == 0), stop=(et == ET - 1))
                    nc.scalar.activation(rT[:, fm, :], zp[:], AF.Relu,
                                         bias=b1[:, fm:fm + 1])

                def ffn2_qq(half, qq):
                    rT = rT_tiles[half]
                    qm = half * 4 + qq
                    ot = outp.tile([128, E], f32)
                    op = ops.tile([128, 1024], f32)
                    for ec in range(2):
                        for fk in range(FT):
                            nc.tensor.matmul(
                                op[:, ec * 512:(ec + 1) * 512],
                                lhsT=rT[:, fk, qq * 128:(qq + 1) * 128],
                                rhs=w2_sb[:, fk, ec * 512:(ec + 1) * 512],
                                start=(fk == 0), stop=(fk == FT - 1))
                    nc.vector.tensor_add(ot[:], op[:], xmid[:, qm, :])
                    nc.scalar.dma_start(out[qm * 128:(qm + 1) * 128, :], ot[:])

                for qm in range(4):
                    ln2_qm(qm)
                # LN2 chunks 4..7 drip between the first 4 fm-groups of
                # FFN1-half0 (which only needs ln2T token cols 0:512).
                for g in range(4):
                    ln2_qm(4 + g)
                    for fm in range(g * 8, g * 8 + 8):
                        ffn1_fm(0, fm)
                for qq in range(4):
                    ffn2_qq(0, qq)
                rT_tiles.pop(0)
                for fm in range(FT):
                    ffn1_fm(1, fm)
                for qq in range(4):
                    ffn2_qq(1, qq)

    nc.compile()
    _CACHE[key] = nc
    return nc


def _prep_inputs(x, wq, wk, wv, proj_w, proj_b, g1, beta1, g2, beta2, w1, bf1,
                 w2, bf2):
    """Host-side sharding + weight folding. Returns list of 8 in_maps."""
    f32 = np.float32
    x = np.asarray(x, f32)
    scale = float(E) ** -0.5

    Wq = np.asarray(wq, f32).transpose(1, 0, 2).reshape(E, H * D) * scale
    Wk = np.asarray(wk, f32).transpose(1, 0, 2).reshape(E, H * D)
    Wv = np.asarray(wv, f32).transpose(1, 0, 2).reshape(E, H * D)
    g1 = np.asarray(g1, f32)
    beta1 = np.asarray(beta1, f32)
    g2 = np.asarray(g2, f32)
    beta2 = np.asarray(beta2, f32)
    w1 = np.asarray(w1, f32)
    w2 = np.asarray(w2, f32)
    bf1 = np.asarray(bf1, f32)
    bf2 = np.asarray(bf2, f32)
    proj_w = np.asarray(proj_w, f32)
    proj_b = np.asarray(proj_b, f32)

    def sb_layout(w, ntile):
        # [ntile*128, N] -> [128, ntile*N] with per-partition contiguous tiles
        n = w.shape[1]
        return np.ascontiguousarray(
            w.reshape(ntile, 128, n).transpose(1, 0, 2).reshape(128, ntile * n))

    wq_b = sb_layout((Wq * g1[:, None]).astype(BF16), ET)
    wk_b = sb_layout((Wk * g1[:, None]).astype(BF16), ET)
    wv_b = sb_layout((Wv * g1[:, None]).astype(BF16), ET)
    qbias = beta1 @ Wq
    kbias = beta1 @ Wk
    vbias = beta1 @ Wv
    w1_b = np.ascontiguousarray(
        (w1 * g2[:, None]).astype(BF16)
        .reshape(ET, 128, FT, 128).transpose(1, 2, 0, 3)
        .reshape(128, FT * ET * 128))
    b1v = bf1 + beta2 @ w1
    w2_b = sb_layout(w2.astype(BF16), FT)
    projw_b = sb_layout(proj_w.astype(BF16), NP)

    qb = np.ascontiguousarray(qbias.reshape(ET, 128).T, f32)
    kb = np.ascontiguousarray(kbias.reshape(ET, 128).T, f32)
    vb = np.ascontiguousarray(np.broadcast_to(vbias, (128, H * D))).astype(BF16)
    b1m = np.ascontiguousarray(b1v.reshape(FT, 128).T, f32)
    bf2m = np.ascontiguousarray(np.broadcast_to(bf2, (128, E))).astype(BF16)

    tri = np.triu(np.ones((128, 128), f32))  # [k_row, q_col]: 1 iff k <= q
    onesm = np.ones((128, 128), f32)
    zerosm = np.zeros((128, 128), f32)
    # maskE = causal diagonal (all cores); maskO = parity: the pair-core's
    # diagonal-adjacent chunk is strictly past (odd cores) or future (even).
    mO = {0: zerosm, 1: tri * 0 + 1}

    in_maps = []
    for c in range(NCORES):
        b, hpar = c // 2, c % 2
        xc = x[b].reshape(TK, 128, E)
        xq = np.ascontiguousarray(xc[hpar::2].reshape(M, E), f32)
        xperm = np.ascontiguousarray(
            np.concatenate([xc[hpar::2], xc[1 - hpar::2]], axis=0)
            .reshape(T, E)).astype(BF16)
        in_maps.append({
            "x_full": xperm,
            "xqp": xq + proj_b[None, :].astype(f32),
            "wq": wq_b, "wk": wk_b, "wv": wv_b,
            "projw": projw_b, "w1": w1_b, "w2": w2_b,
            "qb": qb, "kb": kb, "vb": vb, "b1": b1m, "bf2b": bf2m,
            "maskE": np.ascontiguousarray(
                np.tile(tri, (1, 2))).astype(BF16),
            "maskO": np.ascontiguousarray(
                np.tile(mO[hpar], (1, 2))).astype(BF16),
        })
    return in_maps


def _run(inputs, trace=False):
    from concourse.bass_utils import run_bass_kernel_spmd
    nc = _build()
    in_maps = _prep_inputs(**inputs)
    res = run_bass_kernel_spmd(nc, in_maps, core_ids=list(range(NCORES)),
                               trace=trace)
    full = np.empty((B, T, E), np.float32)
    for c in range(NCORES):
        b, hpar = c // 2, c % 2
        full[b].reshape(TK, 128, E)[hpar::2] = (
            res.results[c]["out"].reshape(NS, 128, E))
    return full, res


def kernel(**inputs) -> np.ndarray:
    out, _ = _run(inputs, trace=False)
    return out



# revision 25
# speedup vs baseline: 1.1296x; 1.0501x over previous
# Trainium2 Bass kernel for nn_DecoderBlock (B=4, T=2048, E=1024, H=16, D=64, FF=4096).
#
# Sharding: 8-way data parallel, zero collectives. Core c = 2*b + h handles batch b
# and the interleaved half of the sequence: 128-row q-blocks {2s+h : s=0..7}
# (1024 q rows per core). K/V are computed per-core for the full T=2048 rows of its
# batch (duplicated across the two cores of a batch pair) so attention needs no
# cross-core communication. The interleaved block assignment makes the causal
# work pattern identical on every core (uniform SPMD program): q-slot s statically
# attends keys [0, 256*(s+1)), with a per-core {0,1} multiplicative mask (input
# data) handling the parity-dependent diagonal.
#
# On-chip layout: activations flow feature-major (S^T = [k, q]) through attention so
# softmax needs no transposes of the probability matrix. Softmax uses no max
# subtraction (scores are ~N(0, 0.25^2) by construction); 1/denominator is
# exp(-ln(d)) so the whole kernel uses one ACT table set (exp/ln/relu).
# Matmuls run in bf16 with fp32 PSUM accumulation; LN statistics, residuals and
# the final output stay fp32. LN gains (g1,g2) and the attention 1/sqrt(E) scale
# are folded into the weights on the host; beta terms become per-feature biases.

import numpy as np
import ml_dtypes
from contextlib import ExitStack

BF16 = ml_dtypes.bfloat16

B, T, E, H, D, FF = 4, 2048, 1024, 16, 64, 4096
M = 1024          # q rows per core
NCORES = 8
NS = 8            # q slots (128 rows) per core
ET = E // 128     # 8 e-tiles
TK = T // 128     # 16 k-tiles
FT = FF // 128    # 32 ff-tiles
NP = H // 2       # 8 head pairs
EPS = 1e-5

_CACHE = {}


def _build(repeat=1):
    """Build (and cache) the Bass module for one core's uniform program.

    repeat>1 emits the whole body N times (identical I/O) — used only for
    slope-based wall-clock timing of one body on hardware.
    """
    key = ("nc", repeat)
    if key in _CACHE:
        return _CACHE[key]

    import concourse.bacc as bacc
    import concourse.tile as tile
    import concourse.mybir as mybir
    from concourse import masks as cmasks

    dt = mybir.dt
    f32, bf16 = dt.float32, dt.bfloat16
    AF = mybir.ActivationFunctionType
    OP = mybir.AluOpType

    nc = bacc.Bacc("TRN2", target_bir_lowering=False, debug=False,
                   num_devices=NCORES)

    # Every activation we use (Exp, Ln, Relu, Copy, Identity) lives in the
    # 'natural_log_exp_and_others' table set. The default per-function set
    # choice alternates home sets (exp_and_others vs natural_log), inserting
    # ~80 ACT table loads (~100us). Restrict the chooser to the one set that
    # covers everything -> a single load.
    import types
    import bass_rust as _br

    def _insert_act_loads_one_set(self):
        has_activation = any(
            isinstance(i, mybir.InstActivation)
            for b in self.main_func.blocks for i in b.instructions)
        if not has_activation:
            return
        tabs = bacc.get_activation_tables(self.m.arch)
        ours = {mybir.ActivationFunctionType.Exp, mybir.ActivationFunctionType.Ln,
                mybir.ActivationFunctionType.Relu, mybir.ActivationFunctionType.Copy,
                mybir.ActivationFunctionType.Identity}
        filt = []
        for name, fns in tabs.items():
            if name == "natural_log_exp_and_others":
                assert ours <= fns
                filt.append((name, fns))
            else:
                filt.append((name, fns - ours))
        _br.insert_act_table_loads(self, filt)

    nc.insert_act_table_loads = types.MethodType(_insert_act_loads_one_set, nc)

    # ----- DRAM I/O -----
    x_full = nc.dram_tensor("x_full", [T, E], bf16, kind="ExternalInput").ap()
    xqp = nc.dram_tensor("xqp", [M, E], f32, kind="ExternalInput").ap()
    # weights arrive pre-arranged on the host into SBUF layout
    # [128 partitions, <tile dims>] so each DMA is one long contiguous run
    # per partition (minimal descriptor count).
    wq = nc.dram_tensor("wq", [128, ET * H * D], bf16, kind="ExternalInput").ap()
    wk = nc.dram_tensor("wk", [128, ET * H * D], bf16, kind="ExternalInput").ap()
    wv = nc.dram_tensor("wv", [128, ET * H * D], bf16, kind="ExternalInput").ap()
    projw = nc.dram_tensor("projw", [128, NP * E], bf16, kind="ExternalInput").ap()
    w1 = nc.dram_tensor("w1", [128, FT * ET * 128], bf16,
                        kind="ExternalInput").ap()
    w2 = nc.dram_tensor("w2", [128, FT * E], bf16, kind="ExternalInput").ap()
    qb_d = nc.dram_tensor("qb", [128, ET], f32, kind="ExternalInput").ap()
    kb_d = nc.dram_tensor("kb", [128, ET], f32, kind="ExternalInput").ap()
    vb_d = nc.dram_tensor("vb", [128, H * D], bf16, kind="ExternalInput").ap()
    b1_d = nc.dram_tensor("b1", [128, FT], f32, kind="ExternalInput").ap()
    bf2_d = nc.dram_tensor("bf2b", [128, E], bf16, kind="ExternalInput").ap()
    maskE_d = nc.dram_tensor("maskE", [128, 256], bf16, kind="ExternalInput").ap()
    maskO_d = nc.dram_tensor("maskO", [128, 256], bf16, kind="ExternalInput").ap()
    out = nc.dram_tensor("out", [M, E], f32, kind="ExternalOutput").ap()

    with tile.TileContext(nc) as tc:
      for _rep in range(repeat):
        es = ExitStack()
        with es:
            # ---------- constants (whole kernel) ----------
            constp = es.enter_context(tc.tile_pool(name="const", bufs=1))
            ident = constp.tile([128, 128], bf16)
            cmasks.make_identity(nc, ident[:])
            maskE = constp.tile([128, 256], bf16)
            nc.sync.dma_start(maskE[:], maskE_d)
            maskO = constp.tile([128, 256], bf16)
            nc.sync.dma_start(maskO[:], maskO_d)
            qb = constp.tile([128, ET], f32)
            nc.sync.dma_start(qb[:], qb_d)
            kb = constp.tile([128, ET], f32)
            nc.sync.dma_start(kb[:], kb_d)
            vb = constp.tile([128, H * D], bf16)
            nc.sync.dma_start(vb[:], vb_d)
            b1 = constp.tile([128, FT], f32)
            nc.sync.dma_start(b1[:], b1_d)
            bf2 = constp.tile([128, E], bf16)
            nc.sync.dma_start(bf2[:], bf2_d)
            eps_t = constp.tile([128, 1], f32)
            nc.gpsimd.memset(eps_t[:], EPS)

            # helper: layernorm one 128-row chunk (fp32 src slice in SBUF) and
            # write the transposed bf16 result into dst_T[:, et, col:col+128].
            def ln_stats(src, statp):
                st = statp.tile([128, 2, 6], f32, tag="st")
                for g in range(2):
                    nc.vector.bn_stats(st[:, g, :], src[:, g * 512:(g + 1) * 512])
                ag = statp.tile([128, 2], f32, tag="ag")
                nc.vector.bn_aggr(ag[:], st[:])
                lv = statp.tile([128, 1], f32, tag="lv")
                nc.scalar.activation(lv[:], ag[:, 1:2], AF.Ln, bias=eps_t[:])
                rstd = statp.tile([128, 1], f32, tag="rstd")
                nc.scalar.activation(rstd[:], lv[:], AF.Exp, scale=-0.5)
                return ag, rstd

            def ln_chunk(src, dst_T, col, statp, lnstage, tpsum, ci,
                         stats=None):
                ag, rstd = stats if stats is not None else ln_stats(src, statp)
                lc = lnstage.tile([128, E], bf16)
                # split the normalize-apply across DVE and GPSIMD so the
                # per-chunk chain latency halves and both engines share work
                nc.vector.tensor_scalar(lc[:, 0:512], src[:, 0:512],
                                        ag[:, 0:1], rstd[:],
                                        OP.subtract, OP.mult)
                nc.gpsimd.tensor_scalar(lc[:, 512:1024], src[:, 512:1024],
                                        ag[:, 0:1], rstd[:],
                                        OP.subtract, OP.mult)
                import os as _os
                if _os.environ.get("KT_DMA_TRANSPOSE", "0") == "1":
                    for et in range(ET):
                        nc.sync.dma_start(dst_T[:, et, col:col + 128],
                                            lc[:, et * 128:(et + 1) * 128],
                                            transpose=True)
                else:
                    for et in range(ET):
                        tp = tpsum.tile([128, 128], bf16)
                        nc.tensor.transpose(tp[:],
                                            lc[:, et * 128:(et + 1) * 128],
                                            ident[:])
                        dst = dst_T[:, et, col:col + 128]
                        if (et + ci) % 2 == 0:
                            nc.vector.tensor_copy(dst, tp[:])
                        else:
                            nc.scalar.copy(dst, tp[:])
                        # (copies stay off GPSIMD: it cannot read PSUM)

            # ---------- scope B: qT/kT/v (strict stack nesting) ----------
            xmid = es.enter_context(tc.tile_pool(name="xmidp", bufs=1)).tile(
                [128, NS, E], bf16)
            with ExitStack() as sB:
                qT = sB.enter_context(tc.tile_pool(name="qTp", bufs=1)).tile(
                    [128, NP, M], bf16)
                kT = sB.enter_context(tc.tile_pool(name="kTp", bufs=1)).tile(
                    [128, NP, T], bf16)
                # per-pair AV stationary operand [V_h0 | ones(64) | V_h1]:
                # head0 reads cols 0:128, head1 cols 64:192 — the shared ones
                # block makes the same matmul that accumulates attn@V also
                # accumulate the softmax denominator (replicated on the 64
                # out-rows opposite each head's data rows).
                vS = sB.enter_context(tc.tile_pool(name="vp", bufs=1)).tile(
                    [128, TK, NP, 192], bf16)
                nc.gpsimd.memset(vS[:, :, :, 64:128], 1.0)

                # ---------- scope A: LN1 + QKV projections ----------
                with ExitStack() as sA:
                    wpool = sA.enter_context(tc.tile_pool(name="wpool", bufs=1))
                    stage = sA.enter_context(tc.tile_pool(name="xstage", bufs=2))
                    lnstage = sA.enter_context(tc.tile_pool(name="lnstage", bufs=2))
                    statp = sA.enter_context(tc.tile_pool(name="statp", bufs=6))
                    tpsum = sA.enter_context(
                        tc.tile_pool(name="tpsum", bufs=4, space="PSUM"))
                    qps = sA.enter_context(
                        tc.tile_pool(name="qps", bufs=2, space="PSUM"))

                    lnf = sA.enter_context(tc.tile_pool(name="lnfp", bufs=1)).tile(
                        [128, ET, T], bf16)
                    # LN1 over x_full (host-permuted: q rows are chunks 0..7,
                    # the pair-core's rows are chunks 8..15) -> lnf. Q^T, each
                    # K^T token-quarter and each V chunk are emitted as soon
                    # as the lnf region they read is complete, so the PE
                    # works through projections while the LN chains run.
                    wq_sb = wpool.tile([128, ET, H * D], bf16, tag="wq")
                    wk_sb = wpool.tile([128, ET, H * D], bf16, tag="wk")
                    wv_sb = wpool.tile([128, ET, H * D], bf16, tag="wv")
                    nc.sync.dma_start(wv_sb[:], wv.rearrange(
                        "p (et n) -> p et n", et=ET))

                    def v_chunk(t):
                        ps = qps.tile([128, 1024], f32)
                        for hc in range(2):
                            for et in range(ET):
                                nc.tensor.matmul(
                                    ps[:, hc * 512:(hc + 1) * 512],
                                    lhsT=lnf[:, et, t * 128:(t + 1) * 128],
                                    rhs=wv_sb[:, et, hc * 512:(hc + 1) * 512],
                                    start=(et == 0), stop=(et == ET - 1))
                        psv = ps[:].rearrange("p (np h d) -> p np h d", np=NP,
                                              h=2)
                        vbv = vb[:].rearrange("p (np h d) -> p np h d", np=NP,
                                              h=2)
                        nc.vector.tensor_add(
                            vS[:, t, :, 0:64], psv[:, :, 0, :], vbv[:, :, 0, :])
                        nc.vector.tensor_add(
                            vS[:, t, :, 128:192], psv[:, :, 1, :],
                            vbv[:, :, 1, :])

                    def k_quarter(tq):
                        for m in range(ET):
                            ps = qps.tile([128, 512], f32)
                            for et in range(ET):
                                nc.tensor.matmul(
                                    ps[:],
                                    lhsT=wk_sb[:, et, m * 128:(m + 1) * 128],
                                    rhs=lnf[:, et, tq * 512:(tq + 1) * 512],
                                    start=(et == 0), stop=(et == ET - 1))
                            nc.vector.tensor_scalar_add(
                                kT[:, m, tq * 512:(tq + 1) * 512], ps[:],
                                kb[:, m:m + 1])

                    def q_proj():
                        for m in range(ET):
                            ps = qps.tile([128, 1024], f32)
                            for qc in range(2):
                                for et in range(ET):
                                    nc.tensor.matmul(
                                        ps[:, qc * 512:(qc + 1) * 512],
                                        lhsT=wq_sb[:, et,
                                                   m * 128:(m + 1) * 128],
                                        rhs=lnf[:, et,
                                                qc * 512:(qc + 1) * 512],
                                        start=(et == 0), stop=(et == ET - 1))
                            nc.vector.tensor_scalar_add(
                                qT[:, m, :], ps[:], qb[:, m:m + 1])

                    for c in range(TK):
                        xc = stage.tile([128, E], bf16)
                        nc.scalar.dma_start(
                            xc[:], x_full[c * 128:(c + 1) * 128, :])
                        ln_chunk(xc[:], lnf, c * 128, statp, lnstage,
                                 tpsum, c)
                        if c == 0:
                            nc.sync.dma_start(wk_sb[:], wk.rearrange(
                                "p (et n) -> p et n", et=ET))
                        if c == 1:
                            nc.sync.dma_start(wq_sb[:], wq.rearrange(
                                "p (et n) -> p et n", et=ET))
                        v_chunk(c)
                        if c % 4 == 3:
                            k_quarter(c // 4)
                        if c == NS - 1:
                            q_proj()

                # ---------- attention + output projection ----------
                # half-outer: all pairs finish q-cols [0,512) first; the
                # output projection for q-chunks 0..3 is then dripped one
                # chunk at a time between second-half pairs so the PE works
                # through proj while ACT drains the exp/normalize backlog.
                with ExitStack() as sC:
                    oT = sC.enter_context(tc.tile_pool(name="oTp", bufs=1)).tile(
                        [128, NP, M], bf16)
                    ptp = sC.enter_context(tc.tile_pool(name="ptp", bufs=5))
                    normp = sC.enter_context(tc.tile_pool(name="normp", bufs=2))
                    xqpp = sC.enter_context(tc.tile_pool(name="xqpp", bufs=1))
                    pwp = sC.enter_context(tc.tile_pool(name="pwp", bufs=1))
                    apsum = sC.enter_context(
                        tc.tile_pool(name="apsum", bufs=2, space="PSUM"))
                    spsum = sC.enter_context(
                        tc.tile_pool(name="spsum", bufs=2, space="PSUM"))

                    maskEv = maskE[:].rearrange("p (h q) -> p h q", h=2)
                    maskOv = maskO[:].rearrange("p (h q) -> p h q", h=2)

                    pw_sb = pwp.tile([128, NP, E], bf16)
                    nc.sync.dma_start(pw_sb[:], projw.rearrange(
                        "p (m e) -> p m e", m=NP))
                    xq_t = {}

                    def do_norm(av, p, half):
                        # den sits on the 64 rows opposite each head's data;
                        # 1/den = exp(-ln(den)) stays partition-aligned, the
                        # multiply crosses bases (PSUM+SBUF operands).
                        tln = normp.tile([128, 2, 512], f32, tag="tln")
                        rcp = normp.tile([128, 2, 512], bf16, tag="rcp")
                        colr = slice(512 * half, 512 * (half + 1))
                        for h in range(2):
                            dn = slice(64 * (1 - h), 64 * (1 - h) + 64)
                            nc.scalar.activation(tln[dn, h, :],
                                                 av[dn, h, :], AF.Ln)
                            nc.scalar.activation(rcp[dn, h, :],
                                                 tln[dn, h, :], AF.Exp,
                                                 scale=-1.0)
                            nc.vector.tensor_mul(
                                oT[64 * h:64 * h + 64, p, colr],
                                av[64 * h:64 * h + 64, h, :],
                                rcp[dn, h, :])

                    # permuted key order: chunks 0..7 are this core's parity
                    # (incl. the causal diagonal), 8..15 the pair-core's
                    # (strictly past or future, selected by the all-ones/
                    # all-zero parity mask).
                    CHUNKS = [
                        [(kt, 128 * kt, "tri") for kt in range(4)] +
                        [(kt, 128 * (kt - 8), "par") for kt in range(8, 12)],
                        [(kt, 0, None) for kt in range(4)] +
                        [(kt, 128 * (kt - 4), "tri") for kt in range(4, 8)] +
                        [(kt, 0, None) for kt in range(8, 12)] +
                        [(kt, 128 * (kt - 12), "par") for kt in range(12, 16)],
                    ]

                    def do_av(item):
                        av, p, half, kt, qlo, sp, pt = item
                        for h in range(2):
                            nc.tensor.matmul(
                                av[:, h, qlo:512],
                                lhsT=vS[:, kt, p, 64 * h:64 * h + 128],
                                rhs=pt[:, h, qlo:512],
                                start=(kt == 0), stop=sp,
                                skip_group_check=True)
                        if sp:
                            do_norm(av, p, half)

                    def proj_qm(qm):
                        if qm // 4 not in xq_t:
                            xqh = xqpp.tile([128, 4, E], f32, tag="xq")
                            xq_t[qm // 4] = xqh
                        xqh = xq_t[qm // 4]
                        nc.sync.dma_start(
                            xqh[:, qm % 4, :], xqp[qm * 128:(qm + 1) * 128, :])
                        ps = apsum.tile([128, 1024], f32, tag="av")
                        for ec in range(2):
                            for pk in range(NP):
                                nc.tensor.matmul(
                                    ps[:, ec * 512:(ec + 1) * 512],
                                    lhsT=oT[:, pk, qm * 128:(qm + 1) * 128],
                                    rhs=pw_sb[:, pk, ec * 512:(ec + 1) * 512],
                                    start=(pk == 0), stop=(pk == NP - 1))
                        nc.vector.tensor_add(
                            xmid[:, qm, :], ps[:], xqh[:, qm % 4, :])

                    def attn_half(half, interleave=()):
                        chunks = CHUNKS[half]
                        last_kt = chunks[-1][0]
                        pend = []
                        for p in range(NP):
                            av = apsum.tile([128, 2, 512], f32, tag="av")
                            for kt, qlo, mk in chunks:
                                ps = spsum.tile([128, 2, 512], f32)
                                for h in range(2):
                                    nc.tensor.matmul(
                                        ps[:, h, qlo:512],
                                        lhsT=kT[64 * h:64 * h + 64, p,
                                                kt * 128:(kt + 1) * 128],
                                        rhs=qT[64 * h:64 * h + 64, p,
                                               512 * half + qlo:
                                               512 * (half + 1)],
                                        start=True, stop=True)
                                pt = ptp.tile([128, 2, 512], bf16)
                                nc.scalar.activation(
                                    pt[:, :, qlo:512], ps[:, :, qlo:512],
                                    AF.Exp)
                                if mk is not None:
                                    mkv = maskEv if mk == "tri" else maskOv
                                    nc.vector.tensor_mul(
                                        pt[:, :, qlo:qlo + 128],
                                        pt[:, :, qlo:qlo + 128], mkv)
                                pend.append((av, p, half, kt, qlo,
                                             kt == last_kt, pt))
                                if len(pend) > 3:
                                    do_av(pend.pop(0))
                            if p % 2 == 1 and p // 2 < len(interleave):
                                while pend:
                                    do_av(pend.pop(0))
                                interleave[p // 2]()
                        while pend:
                            do_av(pend.pop(0))

                    attn_half(0)
                    attn_half(1, interleave=[
                        (lambda qm=qm: proj_qm(qm)) for qm in range(4)])
                    for qm in range(4, NS):
                        proj_qm(qm)

            # ---------- scope D: LN2 + FFN ----------
            # LN2 q-chunks 0..3 interleave with the tail projections; FFN1's
            # first-half fm groups interleave with LN2 chunks 4..7 so the
            # PE never drains while DVE/ACT run the layernorm chains.
            with ExitStack() as sD:
                ln2T = sD.enter_context(tc.tile_pool(name="ln2p", bufs=1)).tile(
                    [128, ET, M], bf16)
                w2_sb = sD.enter_context(tc.tile_pool(name="w2p", bufs=1)).tile(
                    [128, FT, E], bf16)

                statp2 = sD.enter_context(tc.tile_pool(name="statp2", bufs=6))
                lnstage2 = sD.enter_context(
                    tc.tile_pool(name="lnstage2", bufs=3))
                tpsum2 = sD.enter_context(
                    tc.tile_pool(name="tpsum2", bufs=2, space="PSUM"))
                rtp = sD.enter_context(tc.tile_pool(name="rtp", bufs=1))
                w1p = sD.enter_context(tc.tile_pool(name="w1p", bufs=4))
                zps = sD.enter_context(
                    tc.tile_pool(name="zps", bufs=2, space="PSUM"))
                ops = sD.enter_context(
                    tc.tile_pool(name="ops", bufs=2, space="PSUM"))
                outp = sD.enter_context(tc.tile_pool(name="outp", bufs=3))

                def ln2_qm(qm):
                    ln_chunk(xmid[:, qm, :], ln2T, qm * 128, statp2,
                             lnstage2, tpsum2, qm)
                    # after LN2 consumed xmid, fold the final bf2 bias in
                    nc.vector.tensor_add(xmid[:, qm, :], xmid[:, qm, :],
                                         bf2[:])

                rT_tiles = {}

                def ffn1_fm(half, fm):
                    if half not in rT_tiles:
                        rT = rtp.tile([128, FT, 512], bf16, tag="rT")
                        rT_tiles[half] = rT
                    rT = rT_tiles[half]
                    w1f = w1p.tile([128, ET, 128], bf16)
                    nc.sync.dma_start(
                        w1f[:], w1.rearrange("p (fm et f) -> p fm et f",
                                             fm=FT, et=ET)[:, fm])
                    zp = zps.tile([128, 512], f32)
                    for et in range(ET):
                        nc.tensor.matmul(
                            zp[:],
                            lhsT=w1f[:, et, :],
                            rhs=ln2T[:, et, half * 512:(half + 1) * 512],
                            start=(et == 0), stop=(et == ET - 1))
                    nc.scalar.activation(rT[:, fm, :], zp[:], AF.Relu,
                                         bias=b1[:, fm:fm + 1])

                def ffn2_qq(half, qq):
                    rT = rT_tiles[half]
                    qm = half * 4 + qq
                    ot = outp.tile([128, E], f32)
                    op = ops.tile([128, 1024], f32)
                    for ec in range(2):
                        for fk in range(FT):
                            nc.tensor.matmul(
                                op[:, ec * 512:(ec + 1) * 512],
                                lhsT=rT[:, fk, qq * 128:(qq + 1) * 128],
                                rhs=w2_sb[:, fk, ec * 512:(ec + 1) * 512],
                                start=(fk == 0), stop=(fk == FT - 1))
                    nc.vector.tensor_add(ot[:], op[:], xmid[:, qm, :])
                    nc.scalar.dma_start(out[qm * 128:(qm + 1) * 128, :], ot[:])

                for qm in range(4):
                    ln2_qm(qm)
                # LN2 chunks 4..7 drip between the first 4 fm-groups of
                # FFN1-half0 (which only needs ln2T token cols 0:512).
                for g in range(4):
                    ln2_qm(4 + g)
                    nc.sync.dma_start(
                        w2_sb[:, g * 8:(g + 1) * 8, :],
                        w2.rearrange("p (ft e) -> p ft e",
                                     ft=FT)[:, g * 8:(g + 1) * 8, :])
                    for fm in range(g * 8, g * 8 + 8):
                        ffn1_fm(0, fm)
                for qq in range(4):
                    ffn2_qq(0, qq)
                rT_tiles.pop(0)
                for fm in range(FT):
                    ffn1_fm(1, fm)
                for qq in range(4):
                    ffn2_qq(1, qq)

    nc.compile()
    _CACHE[key] = nc
    return nc


def _prep_inputs(x, wq, wk, wv, proj_w, proj_b, g1, beta1, g2, beta2, w1, bf1,
                 w2, bf2):
    """Host-side sharding + weight folding. Returns list of 8 in_maps."""
    f32 = np.float32
    x = np.asarray(x, f32)
    scale = float(E) ** -0.5

    Wq = np.asarray(wq, f32).transpose(1, 0, 2).reshape(E, H * D) * scale
    Wk = np.asarray(wk, f32).transpose(1, 0, 2).reshape(E, H * D)
    Wv = np.asarray(wv, f32).transpose(1, 0, 2).reshape(E, H * D)
    g1 = np.asarray(g1, f32)
    beta1 = np.asarray(beta1, f32)
    g2 = np.asarray(g2, f32)
    beta2 = np.asarray(beta2, f32)
    w1 = np.asarray(w1, f32)
    w2 = np.asarray(w2, f32)
    bf1 = np.asarray(bf1, f32)
    bf2 = np.asarray(bf2, f32)
    proj_w = np.asarray(proj_w, f32)
    proj_b = np.asarray(proj_b, f32)

    def sb_layout(w, ntile):
        # [ntile*128, N] -> [128, ntile*N] with per-partition contiguous tiles
        n = w.shape[1]
        return np.ascontiguousarray(
            w.reshape(ntile, 128, n).transpose(1, 0, 2).reshape(128, ntile * n))

    wq_b = sb_layout((Wq * g1[:, None]).astype(BF16), ET)
    wk_b = sb_layout((Wk * g1[:, None]).astype(BF16), ET)
    wv_b = sb_layout((Wv * g1[:, None]).astype(BF16), ET)
    qbias = beta1 @ Wq
    kbias = beta1 @ Wk
    vbias = beta1 @ Wv
    w1_b = np.ascontiguousarray(
        (w1 * g2[:, None]).astype(BF16)
        .reshape(ET, 128, FT, 128).transpose(1, 2, 0, 3)
        .reshape(128, FT * ET * 128))
    b1v = bf1 + beta2 @ w1
    w2_b = sb_layout(w2.astype(BF16), FT)
    projw_b = sb_layout(proj_w.astype(BF16), NP)

    qb = np.ascontiguousarray(qbias.reshape(ET, 128).T, f32)
    kb = np.ascontiguousarray(kbias.reshape(ET, 128).T, f32)
    vb = np.ascontiguousarray(np.broadcast_to(vbias, (128, H * D))).astype(BF16)
    b1m = np.ascontiguousarray(b1v.reshape(FT, 128).T, f32)
    bf2m = np.ascontiguousarray(np.broadcast_to(bf2, (128, E))).astype(BF16)

    tri = np.triu(np.ones((128, 128), f32))  # [k_row, q_col]: 1 iff k <= q
    onesm = np.ones((128, 128), f32)
    zerosm = np.zeros((128, 128), f32)
    # maskE = causal diagonal (all cores); maskO = parity: the pair-core's
    # diagonal-adjacent chunk is strictly past (odd cores) or future (even).
    mO = {0: zerosm, 1: tri * 0 + 1}

    in_maps = []
    for c in range(NCORES):
        b, hpar = c // 2, c % 2
        xc = x[b].reshape(TK, 128, E)
        xq = np.ascontiguousarray(xc[hpar::2].reshape(M, E), f32)
        xperm = np.ascontiguousarray(
            np.concatenate([xc[hpar::2], xc[1 - hpar::2]], axis=0)
            .reshape(T, E)).astype(BF16)
        in_maps.append({
            "x_full": xperm,
            "xqp": xq + proj_b[None, :].astype(f32),
            "wq": wq_b, "wk": wk_b, "wv": wv_b,
            "projw": projw_b, "w1": w1_b, "w2": w2_b,
            "qb": qb, "kb": kb, "vb": vb, "b1": b1m, "bf2b": bf2m,
            "maskE": np.ascontiguousarray(
                np.tile(tri, (1, 2))).astype(BF16),
            "maskO": np.ascontiguousarray(
                np.tile(mO[hpar], (1, 2))).astype(BF16),
        })
    return in_maps


def _run(inputs, trace=False):
    from concourse.bass_utils import run_bass_kernel_spmd
    nc = _build()
    in_maps = _prep_inputs(**inputs)
    res = run_bass_kernel_spmd(nc, in_maps, core_ids=list(range(NCORES)),
                               trace=trace)
    full = np.empty((B, T, E), np.float32)
    for c in range(NCORES):
        b, hpar = c // 2, c % 2
        full[b].reshape(TK, 128, E)[hpar::2] = (
            res.results[c]["out"].reshape(NS, 128, E))
    return full, res


def kernel(**inputs) -> np.ndarray:
    out, _ = _run(inputs, trace=False)
    return out

